# revision 11
# baseline (speedup 1.0000x reference)
"""Trainium2 Bass kernel for nn_MeshEdgeBlock (GNN edge-block message passing).

Computes, per edge e with endpoints (s, d):
    x  = concat([src_nodes[s], dst_nodes[d], edge_feat[e]])   # [384]
    h  = silu(x @ W1 + b1)                                    # [512]
    y  = h @ W2 + b2                                          # [128]
    y  = LayerNorm(y) * gamma + beta + edge_feat[e]           # [128]

Sharding: edges (and index arrays) split across the 8 NeuronCores; node
tables and MLP weights replicated.

Gather strategy (the perf-critical part): the per-edge node-row gather uses
the GpSimd dma_gather instruction in transpose mode — ONE SWDGE op fetches
up to 1024 node rows AND lands them feature-on-partition (the exact rhs
layout mm1 needs), vs. the 994ns-overhead-per-128-rows indirect-DMA path.
dma_gather indices are int16 (<=32767), so the 100000-row node tables are
split into 4 chunks of 25000 rows and edges are bucketed host-side by
(src_chunk, dst_chunk) — 16 buckets; each bucket's edges are padded to
whole 128-edge tiles per core and processed in groups of up to 8 tiles
whose gathers read from that bucket's chunk slices. The host permutation
is inverted when assembling the full output.

Device-side dataflow per group (gt<=8 tiles of 128 edges):
  - 2 dma_gathers (src, dst) -> sT/dT [128 feat, gt*128 edges] bf16
  - edge features: plain DMA [128 edge, gt, 128 feat] (residual) plus an
    xbar DMA transpose -> eT [128 feat, gt*128] (mm1 rhs); PE does no
    transposes at all
  - per tile: mm1 12 bf16 matmuls -> hT psum; silu (ScalarE); mm2 4
    matmuls -> y psum; LN stats via bn_stats/bn_aggr (VectorE)
  - rsqrt(var+eps) once per group on VectorE (exponent-bit seed + 2
    Newton steps); normalize + residual fused in one affine_then_add
"""

import numpy as np
import ml_dtypes
from contextlib import ExitStack

import concourse.bass as bass
import concourse.tile as tile
from concourse import bacc, library_config, mybir
from concourse.bass_utils import run_bass_kernel_spmd

# Problem constants (hardcoded per spec)
N_CORES = 8
E_FULL = 250000
N_NODES = 100000
D = 128          # node/edge feature dim == LN dim
H = 512          # hidden dim
LN_EPS = 1e-5

CHUNK = 25000    # node-table chunk rows (int16 gather index range)
NCH = 4          # chunks per table; NCH*CHUNK == N_NODES
NBUCKET = NCH * NCH
GMAX = 16        # max 128-edge tiles per gather group

BF16 = mybir.dt.bfloat16
F32 = mybir.dt.float32
I32 = mybir.dt.int32
I16 = mybir.dt.int16

RSQRT_MAGIC = 0x5F3759DF

# bench bisection: 'full' | 'nogather' (contiguous DMA in place of gathers)
# | 'gatheronly' (gathers + copy-out only) | 'dmaonly' (plain DMA only)
VARIANT = "full"

_PROGRAM_CACHE = {}


def _rsqrt_batched(nc, stats, mg2, gt):
    """inv = rsqrt(var + eps), nmi = -mu * inv, batched over the group.

    mg2: [128, 2*gt] f32 slice with (mean, var) pairs per tile. Returns
    (inv, nmi) [128, gt] views. fp32 exponent-bit seed + two Newton steps
    y <- y*(1.5 + (-veps/2)*y^2); rel err ~5e-6.
    """
    mu = mg2[:, 0:2 * gt:2]
    var = mg2[:, 1:2 * gt:2]
    veps = stats.tile([128, GMAX], F32, tag="veps")
    nc.vector.tensor_scalar(out=veps[:, :gt], in0=var, scalar1=LN_EPS,
                            scalar2=None, op0=mybir.AluOpType.add)
    hv = stats.tile([128, GMAX], F32, tag="hv")
    nc.vector.tensor_scalar(out=hv[:, :gt], in0=veps[:, :gt], scalar1=-0.5,
                            scalar2=None, op0=mybir.AluOpType.mult)
    sh = stats.tile([128, GMAX], I32, tag="sh")
    nc.vector.tensor_scalar(out=sh[:, :gt], in0=veps[:, :gt].bitcast(I32),
                            scalar1=1, scalar2=None,
                            op0=mybir.AluOpType.arith_shift_right)
    seed = stats.tile([128, GMAX], I32, tag="seed")
    nc.vector.tensor_scalar(out=seed[:, :gt], in0=sh[:, :gt], scalar1=-1,
                            scalar2=RSQRT_MAGIC,
                            op0=mybir.AluOpType.mult,
                            op1=mybir.AluOpType.add)
    y = seed[:, :gt].bitcast(F32)
    for it in range(2):
        a = stats.tile([128, GMAX], F32, tag=f"nr_a{it}")
        nc.vector.tensor_mul(out=a[:, :gt], in0=y, in1=y)
        b = stats.tile([128, GMAX], F32, tag=f"nr_b{it}")
        nc.vector.tensor_mul(out=b[:, :gt], in0=a[:, :gt], in1=hv[:, :gt])
        ynew = stats.tile([128, GMAX], F32, tag=f"nr_y{it}")
        nc.vector.scalar_tensor_tensor(out=ynew[:, :gt], in0=b[:, :gt],
                                       scalar=1.5, in1=y,
                                       op0=mybir.AluOpType.add,
                                       op1=mybir.AluOpType.mult)
        y = ynew[:, :gt]
    nmi = stats.tile([128, GMAX], F32, tag="nmi")
    nc.vector.scalar_tensor_tensor(out=nmi[:, :gt], in0=mu, scalar=-1.0,
                                   in1=y, op0=mybir.AluOpType.mult,
                                   op1=mybir.AluOpType.mult)
    return y, nmi[:, :gt]


def _build_program(trivial_affine: bool, gshape, sim_safe: bool = False,
                   repeats: int = 1):
    """Build (and cache) the Bass program for a group layout.

    gshape: tuple of (src_chunk, dst_chunk, gt) per gather group. The
    layout is data-dependent (edges bucketed by node chunk), so programs
    are cached per shape. sim_safe replaces Silu (absent in CoreSim) with
    Sigmoid+mult. repeats>1 wraps the body in a hardware For loop.
    """
    gshape = tuple(gshape)
    key = (trivial_affine, sim_safe, repeats, VARIANT, gshape)
    if key in _PROGRAM_CACHE:
        return _PROGRAM_CACHE[key]
    do_gather = VARIANT in ("full", "gatheronly")
    do_compute = VARIANT in ("full", "nogather")

    NG = len(gshape)
    NT = sum(g[2] for g in gshape)
    EC2 = NT * 128

    nc = bacc.Bacc("TRN2", target_bir_lowering=False, debug=False,
                   num_devices=N_CORES)

    nodes = nc.dram_tensor("nodes", [2 * N_NODES, D], BF16, kind="ExternalInput").ap()
    edges = nc.dram_tensor("edges", [EC2, D], BF16, kind="ExternalInput").ap()
    sidx = nc.dram_tensor("sidx", [NG * 128, GMAX * 8], I16, kind="ExternalInput").ap()
    didx = nc.dram_tensor("didx", [NG * 128, GMAX * 8], I16, kind="ExternalInput").ap()
    w1 = nc.dram_tensor("w1", [D, 12 * D], BF16, kind="ExternalInput").ap()
    w2 = nc.dram_tensor("w2", [D, 4 * D], BF16, kind="ExternalInput").ap()
    out = nc.dram_tensor("out", [EC2, D], F32, kind="ExternalOutput").ap()
    scratch = None
    if VARIANT in ("gatheronly", "dmaonly"):
        scratch = nc.dram_tensor("scratch", [128, 2 * GMAX * 128], BF16).ap()
    if not trivial_affine:
        b1d = nc.dram_tensor("b1d", [D, 4], F32, kind="ExternalInput").ap()
        b2d = nc.dram_tensor("b2d", [D, D], F32, kind="ExternalInput").ap()
        gmd = nc.dram_tensor("gmd", [D, D], F32, kind="ExternalInput").ap()
        btd = nc.dram_tensor("btd", [D, D], F32, kind="ExternalInput").ap()

    with tile.TileContext(nc) as tc, ExitStack() as ctx:
        const = ctx.enter_context(tc.tile_pool(name="const", bufs=1))
        io = ctx.enter_context(tc.tile_pool(name="io", bufs=3))
        idxp = ctx.enter_context(tc.tile_pool(name="idx", bufs=3))
        gtp = ctx.enter_context(tc.tile_pool(name="gt", bufs=3))
        htp = ctx.enter_context(tc.tile_pool(name="ht", bufs=3))
        stats = ctx.enter_context(tc.tile_pool(name="stats", bufs=2))
        ps_ht = ctx.enter_context(tc.tile_pool(name="ps_ht", bufs=3, space="PSUM"))
        ps_y = ctx.enter_context(tc.tile_pool(name="ps_y", bufs=3, space="PSUM"))

        if do_gather:
            # dma_gather lives in the dynamically-loaded 'mlp' Q7 library
            nc.gpsimd.load_library(library_config.mlp)

        # constants
        w1sb = const.tile([D, 12 * D], BF16)
        nc.sync.dma_start(out=w1sb[:], in_=w1[:])
        w2sb = const.tile([D, 4 * D], BF16)
        nc.sync.dma_start(out=w2sb[:], in_=w2[:])
        if not trivial_affine:
            b1sb = const.tile([D, 4], F32)
            nc.sync.dma_start(out=b1sb[:], in_=b1d[:])
            b2sb = const.tile([D, D], F32)
            nc.sync.dma_start(out=b2sb[:], in_=b2d[:])
            gmsb = const.tile([D, D], F32)
            nc.sync.dma_start(out=gmsb[:], in_=gmd[:])
            btsb = const.tile([D, D], F32)
            nc.sync.dma_start(out=btsb[:], in_=btd[:])

        def _group(g, sc, dc, gt, toff):
            base = toff * 128
            ne = gt * 128
            if do_gather:
                it_s = idxp.tile([128, GMAX * 8], I16, tag="sidx")
                nc.sync.dma_start(out=it_s[:], in_=sidx[g * 128:(g + 1) * 128, :])
                it_d = idxp.tile([128, GMAX * 8], I16, tag="didx")
                nc.sync.dma_start(out=it_d[:], in_=didx[g * 128:(g + 1) * 128, :])
            sT = gtp.tile([128, 1, GMAX * 128], BF16, tag="sT")
            dT = gtp.tile([128, 1, GMAX * 128], BF16, tag="dT")
            if do_gather:
                nc.gpsimd.dma_gather(
                    sT[:, :, :ne], nodes[sc * CHUNK:(sc + 1) * CHUNK, :],
                    it_s[:, :gt * 8], ne, ne, D, transpose=True,
                    single_packet=False)
                nc.gpsimd.dma_gather(
                    dT[:, :, :ne],
                    nodes[N_NODES + dc * CHUNK:N_NODES + (dc + 1) * CHUNK, :],
                    it_d[:, :gt * 8], ne, ne, D, transpose=True,
                    single_packet=False)
            else:
                # bisect probe: same bytes, contiguous
                b2 = (g * 2 * ne) % (2 * N_NODES - 2 * ne)
                nc.sync.dma_start(
                    out=sT[:, 0, :ne],
                    in_=nodes[b2:b2 + ne, :].rearrange("(p k) f -> p (k f)", p=128))
                nc.sync.dma_start(
                    out=dT[:, 0, :ne],
                    in_=nodes[b2 + ne:b2 + 2 * ne, :].rearrange("(p k) f -> p (k f)", p=128))
            e_bf = io.tile([128, GMAX, D], BF16, tag="e")
            nc.sync.dma_start(
                out=e_bf[:, :gt, :],
                in_=edges[base:base + ne, :].rearrange("(g p) f -> p g f", p=128))
            if do_compute:
                eT = gtp.tile([128, GMAX * 128], BF16, tag="eT")
                nc.sync.dma_start(out=eT[:, :ne], in_=edges[base:base + ne, :],
                                  transpose=True)
                ysb = io.tile([128, GMAX, D], F32, tag="ysb")
                yout = io.tile([128, GMAX, D], F32, tag="yout")
                mg2 = stats.tile([128, 2 * GMAX], F32, tag="mg2")

            for t in range(gt if do_compute else 0):
                co = t * 128
                rhs3 = (sT[:, 0, co:co + 128], dT[:, 0, co:co + 128],
                        eT[:, co:co + 128])
                htps = ps_ht.tile([128, H], F32)
                for m in range(4):
                    for c in range(3):
                        nc.tensor.matmul(
                            out=htps[:, m * D:(m + 1) * D],
                            lhsT=w1sb[:, (c * 4 + m) * D:(c * 4 + m + 1) * D],
                            rhs=rhs3[c],
                            start=(c == 0), stop=(c == 2))

                ht = htp.tile([128, H], BF16)
                if sim_safe:
                    sg = htp.tile([128, H], BF16, tag="sg")
                    nc.scalar.activation(out=sg[:], in_=htps[:],
                                         func=mybir.ActivationFunctionType.Sigmoid)
                    nc.vector.tensor_mul(out=ht[:], in0=htps[:], in1=sg[:])
                    if not trivial_affine:
                        raise NotImplementedError("sim_safe assumes trivial affine")
                elif trivial_affine:
                    nc.scalar.activation(out=ht[:], in_=htps[:],
                                         func=mybir.ActivationFunctionType.Silu)
                else:
                    for m in range(4):
                        nc.scalar.activation(
                            out=ht[:, m * D:(m + 1) * D],
                            in_=htps[:, m * D:(m + 1) * D],
                            func=mybir.ActivationFunctionType.Silu,
                            bias=b1sb[:, m:m + 1])

                yps = ps_y.tile([128, D], F32)
                for m in range(4):
                    nc.tensor.matmul(
                        out=yps[:],
                        lhsT=ht[:, m * D:(m + 1) * D],
                        rhs=w2sb[:, m * D:(m + 1) * D],
                        start=(m == 0), stop=(m == 3))

                if not trivial_affine:
                    nc.vector.tensor_add(out=ysb[:, t, :], in0=yps[:], in1=b2sb[:])
                else:
                    nc.scalar.activation(out=ysb[:, t, :], in_=yps[:],
                                         func=mybir.ActivationFunctionType.Copy)

                st6 = stats.tile([128, 6], F32, tag="st6")
                nc.vector.bn_stats(out=st6[:], in_=ysb[:, t, :])
                nc.vector.bn_aggr(out=mg2[:, 2 * t:2 * t + 2], in_=st6[:])

            if do_compute:
                inv, nmi = _rsqrt_batched(nc, stats, mg2[:, :2 * gt], gt)
            for t in range(gt if do_compute else 0):
                if trivial_affine:
                    nc.vector.affine_then_add(
                        out=yout[:, t, :], in0=ysb[:, t, :], in1=e_bf[:, t, :],
                        scale=inv[:, t:t + 1], bias=nmi[:, t:t + 1])
                else:
                    yn = io.tile([128, D], F32, tag="yn")
                    nc.vector.tensor_scalar(out=yn[:], in0=ysb[:, t, :],
                                            scalar1=inv[:, t:t + 1],
                                            scalar2=nmi[:, t:t + 1],
                                            op0=mybir.AluOpType.mult,
                                            op1=mybir.AluOpType.add)
                    nc.vector.tensor_mul(out=yn[:], in0=yn[:], in1=gmsb[:])
                    nc.vector.tensor_add(out=yn[:], in0=yn[:], in1=btsb[:])
                    nc.vector.tensor_add(out=yout[:, t, :], in0=yn[:], in1=e_bf[:, t, :])

            if do_compute:
                # p-major row order (row = p*gt + t): contiguous per-partition
                # writes; the host un-permutes via perms
                nc.sync.dma_start(
                    out=out[base:base + ne, :].rearrange("(p g) f -> p g f", g=gt),
                    in_=yout[:, :gt, :])
            else:
                nc.sync.dma_start(out=scratch[:, :ne], in_=sT[:, 0, :ne])
                nc.sync.dma_start(
                    out=scratch[:, GMAX * 128:GMAX * 128 + ne], in_=dT[:, 0, :ne])

        def _body():
            toff = 0
            for g, (sc, dc, gt) in enumerate(gshape):
                _group(g, sc, dc, gt, toff)
                toff += gt

        if repeats == 1:
            _body()
        else:
            with tc.For_i(0, repeats, 1):
                _body()

    nc.compile()
    _PROGRAM_CACHE[key] = nc
    return nc


def _prep(inputs):
    """Host-side bucketing + preprocessing.

    Returns (in_maps, trivial, gshape, perm) where perm[core] maps each
    padded per-core edge row to its global edge id (-1 for padding).
    """
    f = {k: np.asarray(v) for k, v in inputs.items()}
    bf = ml_dtypes.bfloat16

    nodes = np.concatenate([f["src_node_features"], f["dst_node_features"]],
                           axis=0).astype(bf)

    e_bf = f["edge_features"].astype(bf)
    si = f["src_indices"].astype(np.int64)
    di = f["dst_indices"].astype(np.int64)
    E = e_bf.shape[0]

    src_c = si // CHUNK
    dst_c = di // CHUNK
    bucket = (src_c * NCH + dst_c).astype(np.int64)
    order = np.argsort(bucket, kind="stable")
    n_b = np.bincount(bucket, minlength=NBUCKET)
    starts = np.concatenate([[0], np.cumsum(n_b)])

    # shared group layout: per bucket, per-core tile count t_b, split into
    # groups of <= GMAX tiles; groups[i] = (bucket, src_chunk, dst_chunk, gt)
    t_b = [int(-(-n // (N_CORES * 128))) for n in n_b]
    groups = []
    for b in range(NBUCKET):
        tb = t_b[b]
        while tb > 0:
            gt = min(GMAX, tb)
            groups.append((b, b // NCH, b % NCH, gt))
            tb -= gt
    gshape = tuple((sc, dc, gt) for (_, sc, dc, gt) in groups)
    NG = len(gshape)
    NT = sum(g[2] for g in gshape)
    EC2 = NT * 128

    # local (in-chunk) int16 gather indices, in global edge order
    sloc = (si - src_c * CHUNK).astype(np.int16)
    dloc = (di - dst_c * CHUNK).astype(np.int16)

    W1 = f["W1"].astype(np.float32)
    W2 = f["W2"].astype(np.float32)
    w1b = np.concatenate(
        [W1[c * D:(c + 1) * D, m * D:(m + 1) * D] for c in range(3) for m in range(4)],
        axis=1).astype(bf)
    w2b = np.concatenate([W2[m * D:(m + 1) * D, :] for m in range(4)], axis=1).astype(bf)

    b1 = f["b1"].astype(np.float32)
    b2 = f["b2"].astype(np.float32)
    gm = f["ln_gamma"].astype(np.float32)
    bt = f["ln_beta"].astype(np.float32)
    trivial = (not b1.any()) and (not b2.any()) and (not bt.any()) and bool(np.all(gm == 1.0))

    # per-core, per-bucket edge allocations: core c takes a t_b[b]*128-sized
    # slab of bucket b's sorted edge list (short/empty at the tail)
    in_maps, perms = [], []
    for core in range(N_CORES):
        e_core = np.zeros((EC2, D), bf)
        p_core = np.full((EC2,), -1, np.int64)
        sidx_b = np.zeros((NG, 128, GMAX * 8), np.int16)
        didx_b = np.zeros((NG, 128, GMAX * 8), np.int16)
        toff = 0       # tile offset in the core's edge stream
        used = [0] * NBUCKET  # edges of bucket b already taken by this core
        for g, (b, sc, dc, gt) in enumerate(groups):
            ne = gt * 128
            q = t_b[b] * 128
            lo = starts[b] + core * q + used[b]
            hi = min(lo + ne, starts[b + 1], starts[b] + (core + 1) * q)
            k = max(hi - lo, 0)
            used[b] += ne
            base = toff * 128
            if k > 0:
                ids = order[lo:lo + k]
                e_core[base:base + k] = e_bf[ids]
                sl = np.zeros((ne,), np.int16)
                dl = np.zeros((ne,), np.int16)
                sl[:k] = sloc[ids]
                dl[:k] = dloc[ids]
                # out rows are written p-major: DRAM row base + p*gt + t holds
                # edge i = t*128 + p of this group
                i = np.arange(k)
                r = (i % 128) * gt + i // 128
                p_core[base + r] = ids
            else:
                sl = np.zeros((ne,), np.int16)
                dl = np.zeros((ne,), np.int16)
            # wrap int16 indices into dma_gather's 16-partition layout,
            # replicated 8x down the 128 partitions (one copy per Q7 core)
            sidx_b[g, :, :gt * 8] = np.tile(sl.reshape(gt * 8, 16).T, (8, 1))
            didx_b[g, :, :gt * 8] = np.tile(dl.reshape(gt * 8, 16).T, (8, 1))
            toff += gt
        assert toff == NT

        m = {
            "nodes": nodes,
            "edges": e_core,
            "sidx": sidx_b.reshape(NG * 128, GMAX * 8),
            "didx": didx_b.reshape(NG * 128, GMAX * 8),
            "w1": w1b,
            "w2": w2b,
        }
        if not trivial:
            m["b1d"] = np.ascontiguousarray(b1.reshape(4, D).T.astype(np.float32))
            m["b2d"] = np.broadcast_to(b2, (D, D)).copy()
            m["gmd"] = np.broadcast_to(gm, (D, D)).copy()
            m["btd"] = np.broadcast_to(bt, (D, D)).copy()
        in_maps.append(m)
        perms.append(p_core)
    return in_maps, trivial, tuple(gshape), perms


def kernel(**inputs) -> np.ndarray:
    in_maps, trivial, gshape, perms = _prep(inputs)
    nc = _build_program(trivial, gshape)
    res = run_bass_kernel_spmd(nc, in_maps, core_ids=list(range(N_CORES)))
    E = np.asarray(inputs["edge_features"]).shape[0]
    out = np.empty((E, D), np.float32)
    for core in range(N_CORES):
        o = res.results[core]["out"]
        p = perms[core]
        valid = p >= 0
        out[p[valid]] = o[valid]
    return out


# revision 18
# speedup vs baseline: 1.0093x; 1.0093x over previous
"""Trainium2 Bass kernel for nn_MeshEdgeBlock (GNN edge-block message passing).

Computes, per edge e with endpoints (s, d):
    x  = concat([src_nodes[s], dst_nodes[d], edge_feat[e]])   # [384]
    h  = silu(x @ W1 + b1)                                    # [512]
    y  = h @ W2 + b2                                          # [128]
    y  = LayerNorm(y) * gamma + beta + edge_feat[e]           # [128]

Sharding: edges (and index arrays) split across the 8 NeuronCores; node
tables and MLP weights replicated.

Gather strategy (the perf-critical part): the per-edge node-row gather uses
the GpSimd dma_gather instruction in transpose mode — ONE SWDGE op fetches
up to 1024 node rows AND lands them feature-on-partition (the exact rhs
layout mm1 needs), vs. the 994ns-overhead-per-128-rows indirect-DMA path.
dma_gather indices are int16 (<=32767), so the 100000-row node tables are
split into 4 chunks of 25000 rows and edges are bucketed host-side by
(src_chunk, dst_chunk) — 16 buckets; each bucket's edges are padded to
whole 128-edge tiles per core and processed in groups of up to 8 tiles
whose gathers read from that bucket's chunk slices. The host permutation
is inverted when assembling the full output.

Device-side dataflow per group (gt<=8 tiles of 128 edges):
  - 2 dma_gathers (src, dst) -> sT/dT [128 feat, gt*128 edges] bf16
  - edge features: plain DMA [128 edge, gt, 128 feat] (residual) plus an
    xbar DMA transpose -> eT [128 feat, gt*128] (mm1 rhs); PE does no
    transposes at all
  - per tile: mm1 12 bf16 matmuls -> hT psum; silu (ScalarE); mm2 4
    matmuls -> y psum; LN stats via bn_stats/bn_aggr (VectorE)
  - rsqrt(var+eps) once per group on VectorE (exponent-bit seed + 2
    Newton steps); normalize + residual fused in one affine_then_add
"""

import numpy as np
import ml_dtypes
from contextlib import ExitStack

import concourse.bass as bass
import concourse.tile as tile
from concourse import bacc, library_config, mybir
from concourse.bass_utils import run_bass_kernel_spmd

# Problem constants (hardcoded per spec)
N_CORES = 8
E_FULL = 250000
N_NODES = 100000
D = 128          # node/edge feature dim == LN dim
H = 512          # hidden dim
LN_EPS = 1e-5

CHUNK = 25000    # node-table chunk rows (int16 gather index range)
NCH = 4          # chunks per table; NCH*CHUNK == N_NODES
NBUCKET = NCH * NCH
GMAX = 16        # max 128-edge tiles per gather group

BF16 = mybir.dt.bfloat16
F32 = mybir.dt.float32
I32 = mybir.dt.int32
I16 = mybir.dt.int16

RSQRT_MAGIC = 0x5F3759DF

# bench bisection: 'full' | 'nogather' (contiguous DMA in place of gathers)
# | 'gatheronly' (gathers + copy-out only) | 'dmaonly' (plain DMA only)
VARIANT = "full"

_PROGRAM_CACHE = {}


def _rsqrt_batched(nc, stats, mg2, gt):
    """inv = rsqrt(var + eps), nmi = -mu * inv, batched over the group.

    mg2: [128, 2*gt] f32 slice with (mean, var) pairs per tile. Returns
    (inv, nmi) [128, gt] views. fp32 exponent-bit seed + two Newton steps
    y <- y*(1.5 + (-veps/2)*y^2); rel err ~5e-6.
    """
    mu = mg2[:, 0:2 * gt:2]
    var = mg2[:, 1:2 * gt:2]
    veps = stats.tile([128, GMAX], F32, tag="veps")
    nc.vector.tensor_scalar(out=veps[:, :gt], in0=var, scalar1=LN_EPS,
                            scalar2=None, op0=mybir.AluOpType.add)
    hv = stats.tile([128, GMAX], F32, tag="hv")
    nc.vector.tensor_scalar(out=hv[:, :gt], in0=veps[:, :gt], scalar1=-0.5,
                            scalar2=None, op0=mybir.AluOpType.mult)
    sh = stats.tile([128, GMAX], I32, tag="sh")
    nc.vector.tensor_scalar(out=sh[:, :gt], in0=veps[:, :gt].bitcast(I32),
                            scalar1=1, scalar2=None,
                            op0=mybir.AluOpType.arith_shift_right)
    seed = stats.tile([128, GMAX], I32, tag="seed")
    nc.vector.tensor_scalar(out=seed[:, :gt], in0=sh[:, :gt], scalar1=-1,
                            scalar2=RSQRT_MAGIC,
                            op0=mybir.AluOpType.mult,
                            op1=mybir.AluOpType.add)
    y = seed[:, :gt].bitcast(F32)
    for it in range(2):
        a = stats.tile([128, GMAX], F32, tag=f"nr_a{it}")
        nc.vector.tensor_mul(out=a[:, :gt], in0=y, in1=y)
        b = stats.tile([128, GMAX], F32, tag=f"nr_b{it}")
        nc.vector.tensor_mul(out=b[:, :gt], in0=a[:, :gt], in1=hv[:, :gt])
        ynew = stats.tile([128, GMAX], F32, tag=f"nr_y{it}")
        nc.vector.scalar_tensor_tensor(out=ynew[:, :gt], in0=b[:, :gt],
                                       scalar=1.5, in1=y,
                                       op0=mybir.AluOpType.add,
                                       op1=mybir.AluOpType.mult)
        y = ynew[:, :gt]
    nmi = stats.tile([128, GMAX], F32, tag="nmi")
    nc.vector.scalar_tensor_tensor(out=nmi[:, :gt], in0=mu, scalar=-1.0,
                                   in1=y, op0=mybir.AluOpType.mult,
                                   op1=mybir.AluOpType.mult)
    return y, nmi[:, :gt]


def _build_program(trivial_affine: bool, gshape, sim_safe: bool = False,
                   repeats: int = 1):
    """Build (and cache) the Bass program for a group layout.

    gshape: tuple of (src_chunk, dst_chunk, gt) per gather group. The
    layout is data-dependent (edges bucketed by node chunk), so programs
    are cached per shape. sim_safe replaces Silu (absent in CoreSim) with
    Sigmoid+mult. repeats>1 wraps the body in a hardware For loop.
    """
    gshape = tuple(gshape)
    key = (trivial_affine, sim_safe, repeats, VARIANT, gshape)
    if key in _PROGRAM_CACHE:
        return _PROGRAM_CACHE[key]
    do_gather = VARIANT in ("full", "gatheronly")
    do_compute = VARIANT in ("full", "nogather")

    NG = len(gshape)
    NT = sum(g[2] for g in gshape)
    EC2 = NT * 128

    nc = bacc.Bacc("TRN2", target_bir_lowering=False, debug=False,
                   num_devices=N_CORES)

    nodes = nc.dram_tensor("nodes", [2 * N_NODES, D], BF16, kind="ExternalInput").ap()
    edges = nc.dram_tensor("edges", [EC2, D], BF16, kind="ExternalInput").ap()
    sidx = nc.dram_tensor("sidx", [NG * 128, GMAX * 8], I16, kind="ExternalInput").ap()
    didx = nc.dram_tensor("didx", [NG * 128, GMAX * 8], I16, kind="ExternalInput").ap()
    w1 = nc.dram_tensor("w1", [D, 12 * D], BF16, kind="ExternalInput").ap()
    w2 = nc.dram_tensor("w2", [D, 4 * D], BF16, kind="ExternalInput").ap()
    out = nc.dram_tensor("out", [EC2, D], BF16, kind="ExternalOutput").ap()
    scratch = None
    if VARIANT in ("gatheronly", "dmaonly"):
        scratch = nc.dram_tensor("scratch", [128, 2 * GMAX * 128], BF16).ap()
    if not trivial_affine:
        b1d = nc.dram_tensor("b1d", [D, 4], F32, kind="ExternalInput").ap()
        b2d = nc.dram_tensor("b2d", [D, D], F32, kind="ExternalInput").ap()
        gmd = nc.dram_tensor("gmd", [D, D], F32, kind="ExternalInput").ap()
        btd = nc.dram_tensor("btd", [D, D], F32, kind="ExternalInput").ap()

    with tile.TileContext(nc) as tc, ExitStack() as ctx:
        const = ctx.enter_context(tc.tile_pool(name="const", bufs=1))
        io = ctx.enter_context(tc.tile_pool(name="io", bufs=3))
        idxp = ctx.enter_context(tc.tile_pool(name="idx", bufs=3))
        gtp = ctx.enter_context(tc.tile_pool(name="gt", bufs=3))
        htp = ctx.enter_context(tc.tile_pool(name="ht", bufs=3))
        stats = ctx.enter_context(tc.tile_pool(name="stats", bufs=2))
        ps_ht = ctx.enter_context(tc.tile_pool(name="ps_ht", bufs=3, space="PSUM"))
        ps_y = ctx.enter_context(tc.tile_pool(name="ps_y", bufs=3, space="PSUM"))

        if do_gather:
            # dma_gather lives in the dynamically-loaded 'mlp' Q7 library
            nc.gpsimd.load_library(library_config.mlp)

        # constants
        w1sb = const.tile([D, 12 * D], BF16)
        nc.sync.dma_start(out=w1sb[:], in_=w1[:])
        w2sb = const.tile([D, 4 * D], BF16)
        nc.sync.dma_start(out=w2sb[:], in_=w2[:])
        if not trivial_affine:
            b1sb = const.tile([D, 4], F32)
            nc.sync.dma_start(out=b1sb[:], in_=b1d[:])
            b2sb = const.tile([D, D], F32)
            nc.sync.dma_start(out=b2sb[:], in_=b2d[:])
            gmsb = const.tile([D, D], F32)
            nc.sync.dma_start(out=gmsb[:], in_=gmd[:])
            btsb = const.tile([D, D], F32)
            nc.sync.dma_start(out=btsb[:], in_=btd[:])

        def _group(g, sc, dc, gt, toff):
            base = toff * 128
            ne = gt * 128
            if do_gather:
                it_s = idxp.tile([128, GMAX * 8], I16, tag="sidx")
                nc.sync.dma_start(out=it_s[:], in_=sidx[g * 128:(g + 1) * 128, :])
                it_d = idxp.tile([128, GMAX * 8], I16, tag="didx")
                nc.sync.dma_start(out=it_d[:], in_=didx[g * 128:(g + 1) * 128, :])
            sT = gtp.tile([128, 1, GMAX * 128], BF16, tag="sT")
            dT = gtp.tile([128, 1, GMAX * 128], BF16, tag="dT")
            if do_gather:
                nc.gpsimd.dma_gather(
                    sT[:, :, :ne], nodes[sc * CHUNK:(sc + 1) * CHUNK, :],
                    it_s[:, :gt * 8], ne, ne, D, transpose=True,
                    single_packet=False)
                nc.gpsimd.dma_gather(
                    dT[:, :, :ne],
                    nodes[N_NODES + dc * CHUNK:N_NODES + (dc + 1) * CHUNK, :],
                    it_d[:, :gt * 8], ne, ne, D, transpose=True,
                    single_packet=False)
            else:
                # bisect probe: same bytes, contiguous
                b2 = (g * 2 * ne) % (2 * N_NODES - 2 * ne)
                nc.sync.dma_start(
                    out=sT[:, 0, :ne],
                    in_=nodes[b2:b2 + ne, :].rearrange("(p k) f -> p (k f)", p=128))
                nc.sync.dma_start(
                    out=dT[:, 0, :ne],
                    in_=nodes[b2 + ne:b2 + 2 * ne, :].rearrange("(p k) f -> p (k f)", p=128))
            e_bf = io.tile([128, GMAX, D], BF16, tag="e")
            nc.sync.dma_start(
                out=e_bf[:, :gt, :],
                in_=edges[base:base + ne, :].rearrange("(g p) f -> p g f", p=128))
            if do_compute:
                eT = gtp.tile([128, GMAX * 128], BF16, tag="eT")
                nc.sync.dma_start(out=eT[:, :ne], in_=edges[base:base + ne, :],
                                  transpose=True)
                ysb = io.tile([128, GMAX, D], F32, tag="ysb")
                yout = io.tile([128, GMAX, D], BF16, tag="yout")
                mg2 = stats.tile([128, 2 * GMAX], F32, tag="mg2")

            for t in range(gt if do_compute else 0):
                co = t * 128
                rhs3 = (sT[:, 0, co:co + 128], dT[:, 0, co:co + 128],
                        eT[:, co:co + 128])
                htps = ps_ht.tile([128, H], F32)
                for m in range(4):
                    for c in range(3):
                        nc.tensor.matmul(
                            out=htps[:, m * D:(m + 1) * D],
                            lhsT=w1sb[:, (c * 4 + m) * D:(c * 4 + m + 1) * D],
                            rhs=rhs3[c],
                            start=(c == 0), stop=(c == 2))

                ht = htp.tile([128, H], BF16)
                if sim_safe:
                    sg = htp.tile([128, H], BF16, tag="sg")
                    nc.scalar.activation(out=sg[:], in_=htps[:],
                                         func=mybir.ActivationFunctionType.Sigmoid)
                    nc.vector.tensor_mul(out=ht[:], in0=htps[:], in1=sg[:])
                    if not trivial_affine:
                        raise NotImplementedError("sim_safe assumes trivial affine")
                elif trivial_affine:
                    nc.scalar.activation(out=ht[:], in_=htps[:],
                                         func=mybir.ActivationFunctionType.Silu)
                else:
                    for m in range(4):
                        nc.scalar.activation(
                            out=ht[:, m * D:(m + 1) * D],
                            in_=htps[:, m * D:(m + 1) * D],
                            func=mybir.ActivationFunctionType.Silu,
                            bias=b1sb[:, m:m + 1])

                yps = ps_y.tile([128, D], F32)
                for m in range(4):
                    nc.tensor.matmul(
                        out=yps[:],
                        lhsT=ht[:, m * D:(m + 1) * D],
                        rhs=w2sb[:, m * D:(m + 1) * D],
                        start=(m == 0), stop=(m == 3))

                if not trivial_affine:
                    nc.vector.tensor_add(out=ysb[:, t, :], in0=yps[:], in1=b2sb[:])
                else:
                    nc.scalar.activation(out=ysb[:, t, :], in_=yps[:],
                                         func=mybir.ActivationFunctionType.Copy)

                st6 = stats.tile([128, 6], F32, tag="st6")
                nc.vector.bn_stats(out=st6[:], in_=ysb[:, t, :])
                nc.vector.bn_aggr(out=mg2[:, 2 * t:2 * t + 2], in_=st6[:])

            if do_compute:
                inv, nmi = _rsqrt_batched(nc, stats, mg2[:, :2 * gt], gt)
            for t in range(gt if do_compute else 0):
                if trivial_affine:
                    nc.vector.affine_then_add(
                        out=yout[:, t, :], in0=ysb[:, t, :], in1=e_bf[:, t, :],
                        scale=inv[:, t:t + 1], bias=nmi[:, t:t + 1])
                else:
                    yn = io.tile([128, D], F32, tag="yn")
                    nc.vector.tensor_scalar(out=yn[:], in0=ysb[:, t, :],
                                            scalar1=inv[:, t:t + 1],
                                            scalar2=nmi[:, t:t + 1],
                                            op0=mybir.AluOpType.mult,
                                            op1=mybir.AluOpType.add)
                    nc.vector.tensor_mul(out=yn[:], in0=yn[:], in1=gmsb[:])
                    nc.vector.tensor_add(out=yn[:], in0=yn[:], in1=btsb[:])
                    nc.vector.tensor_add(out=yout[:, t, :], in0=yn[:], in1=e_bf[:, t, :])

            if do_compute:
                # p-major row order (row = p*gt + t): contiguous per-partition
                # writes; the host un-permutes via perms
                nc.sync.dma_start(
                    out=out[base:base + ne, :].rearrange("(p g) f -> p g f", g=gt),
                    in_=yout[:, :gt, :])
            else:
                nc.sync.dma_start(out=scratch[:, :ne], in_=sT[:, 0, :ne])
                nc.sync.dma_start(
                    out=scratch[:, GMAX * 128:GMAX * 128 + ne], in_=dT[:, 0, :ne])

        def _body():
            toff = 0
            for g, (sc, dc, gt) in enumerate(gshape):
                _group(g, sc, dc, gt, toff)
                toff += gt

        if repeats == 1:
            _body()
        else:
            with tc.For_i(0, repeats, 1):
                _body()

    nc.compile()
    _PROGRAM_CACHE[key] = nc
    return nc


def _prep(inputs):
    """Host-side bucketing + preprocessing.

    Returns (in_maps, trivial, gshape, perm) where perm[core] maps each
    padded per-core edge row to its global edge id (-1 for padding).
    """
    f = {k: np.asarray(v) for k, v in inputs.items()}
    bf = ml_dtypes.bfloat16

    nodes = np.concatenate([f["src_node_features"], f["dst_node_features"]],
                           axis=0).astype(bf)

    e_bf = f["edge_features"].astype(bf)
    si = f["src_indices"].astype(np.int64)
    di = f["dst_indices"].astype(np.int64)
    E = e_bf.shape[0]

    src_c = si // CHUNK
    dst_c = di // CHUNK
    bucket = (src_c * NCH + dst_c).astype(np.int64)
    order = np.argsort(bucket, kind="stable")
    n_b = np.bincount(bucket, minlength=NBUCKET)
    starts = np.concatenate([[0], np.cumsum(n_b)])

    # shared group layout: per bucket, per-core tile count t_b, split into
    # groups of <= GMAX tiles; groups[i] = (bucket, src_chunk, dst_chunk, gt)
    t_b = [int(-(-n // (N_CORES * 128))) for n in n_b]
    groups = []
    for b in range(NBUCKET):
        tb = t_b[b]
        while tb > 0:
            gt = min(GMAX, tb)
            groups.append((b, b // NCH, b % NCH, gt))
            tb -= gt
    gshape = tuple((sc, dc, gt) for (_, sc, dc, gt) in groups)
    NG = len(gshape)
    NT = sum(g[2] for g in gshape)
    EC2 = NT * 128

    # local (in-chunk) int16 gather indices, in global edge order
    sloc = (si - src_c * CHUNK).astype(np.int16)
    dloc = (di - dst_c * CHUNK).astype(np.int16)

    W1 = f["W1"].astype(np.float32)
    W2 = f["W2"].astype(np.float32)
    w1b = np.concatenate(
        [W1[c * D:(c + 1) * D, m * D:(m + 1) * D] for c in range(3) for m in range(4)],
        axis=1).astype(bf)
    w2b = np.concatenate([W2[m * D:(m + 1) * D, :] for m in range(4)], axis=1).astype(bf)

    b1 = f["b1"].astype(np.float32)
    b2 = f["b2"].astype(np.float32)
    gm = f["ln_gamma"].astype(np.float32)
    bt = f["ln_beta"].astype(np.float32)
    trivial = (not b1.any()) and (not b2.any()) and (not bt.any()) and bool(np.all(gm == 1.0))

    # per-core, per-bucket edge allocations: core c takes a t_b[b]*128-sized
    # slab of bucket b's sorted edge list (short/empty at the tail)
    in_maps, perms = [], []
    for core in range(N_CORES):
        e_core = np.zeros((EC2, D), bf)
        p_core = np.full((EC2,), -1, np.int64)
        sidx_b = np.zeros((NG, 128, GMAX * 8), np.int16)
        didx_b = np.zeros((NG, 128, GMAX * 8), np.int16)
        toff = 0       # tile offset in the core's edge stream
        used = [0] * NBUCKET  # edges of bucket b already taken by this core
        for g, (b, sc, dc, gt) in enumerate(groups):
            ne = gt * 128
            q = t_b[b] * 128
            lo = starts[b] + core * q + used[b]
            hi = min(lo + ne, starts[b + 1], starts[b] + (core + 1) * q)
            k = max(hi - lo, 0)
            used[b] += ne
            base = toff * 128
            if k > 0:
                ids = order[lo:lo + k]
                e_core[base:base + k] = e_bf[ids]
                sl = np.zeros((ne,), np.int16)
                dl = np.zeros((ne,), np.int16)
                sl[:k] = sloc[ids]
                dl[:k] = dloc[ids]
                # out rows are written p-major: DRAM row base + p*gt + t holds
                # edge i = t*128 + p of this group
                i = np.arange(k)
                r = (i % 128) * gt + i // 128
                p_core[base + r] = ids
            else:
                sl = np.zeros((ne,), np.int16)
                dl = np.zeros((ne,), np.int16)
            # wrap int16 indices into dma_gather's 16-partition layout,
            # replicated 8x down the 128 partitions (one copy per Q7 core)
            sidx_b[g, :, :gt * 8] = np.tile(sl.reshape(gt * 8, 16).T, (8, 1))
            didx_b[g, :, :gt * 8] = np.tile(dl.reshape(gt * 8, 16).T, (8, 1))
            toff += gt
        assert toff == NT

        m = {
            "nodes": nodes,
            "edges": e_core,
            "sidx": sidx_b.reshape(NG * 128, GMAX * 8),
            "didx": didx_b.reshape(NG * 128, GMAX * 8),
            "w1": w1b,
            "w2": w2b,
        }
        if not trivial:
            m["b1d"] = np.ascontiguousarray(b1.reshape(4, D).T.astype(np.float32))
            m["b2d"] = np.broadcast_to(b2, (D, D)).copy()
            m["gmd"] = np.broadcast_to(gm, (D, D)).copy()
            m["btd"] = np.broadcast_to(bt, (D, D)).copy()
        in_maps.append(m)
        perms.append(p_core)
    return in_maps, trivial, tuple(gshape), perms


def kernel(**inputs) -> np.ndarray:
    in_maps, trivial, gshape, perms = _prep(inputs)
    nc = _build_program(trivial, gshape)
    res = run_bass_kernel_spmd(nc, in_maps, core_ids=list(range(N_CORES)))
    E = np.asarray(inputs["edge_features"]).shape[0]
    out = np.empty((E, D), np.float32)
    for core in range(N_CORES):
        o = np.asarray(res.results[core]["out"]).astype(np.float32)
        p = perms[core]
        valid = p >= 0
        out[p[valid]] = o[valid]
    return out


# revision 21
# speedup vs baseline: 1.0673x; 1.0575x over previous
"""Trainium2 Bass kernel for nn_MeshEdgeBlock (GNN edge-block message passing).

Computes, per edge e with endpoints (s, d):
    x  = concat([src_nodes[s], dst_nodes[d], edge_feat[e]])   # [384]
    h  = silu(x @ W1 + b1)                                    # [512]
    y  = h @ W2 + b2                                          # [128]
    y  = LayerNorm(y) * gamma + beta + edge_feat[e]           # [128]

Sharding: edges (and index arrays) split evenly across the 8 NeuronCores;
node-feature tables and MLP weights replicated to every core.

Device-side dataflow per core (EC = 31360 padded edges, 245 tiles of 128
edges, grouped G=7 tiles per "supertile"):
  - one indirect-DMA gather per supertile pulls src+dst node rows (bf16)
    from a host-concatenated [2N, 128] table (dst indices offset by N)
  - per 128-edge tile, DMA(xbar)-transpose the three 128x128 bf16 feature
    blocks into xT (features-on-partitions) for the matmuls
  - mm1: 12 bf16 128^3 matmuls -> hT psum bank [128h, 4 chunks x 128 edges]
  - silu on ScalarE in one pass over the bank (b1 == 0 fast path)
  - mm2: 4 bf16 matmuls -> y[128 edges, 128] psum (fp32)
  - LN stats per tile via bn_stats/bn_aggr (VectorE); y copied to SBUF by
    ScalarE (Copy - same activation table set as Silu, so no table reloads)
  - rsqrt(var+eps) computed once per supertile on VectorE with the
    exponent-bit seed + 2 Newton steps (no Sqrt on ScalarE: sqrt lives in a
    different activation-function table set and each switch reloads tables)
  - normalize + residual fused into one custom-DVE affine_then_add:
    out = (y * inv + (-mu*inv)) + edge_feat

Numerics: matmul inputs bf16 (fp32 PSUM accumulation); LN + residual fp32
except edge features, which stay bf16 end-to-end. b1/b2/gamma/beta get a
fast path when they hold the trivial values hardcoded by the problem's
setup_inputs (zeros/ones) - verified on the host per call; non-trivial
values take extra (exact, slightly slower) ops.
"""

import numpy as np
import ml_dtypes
from contextlib import ExitStack

import concourse.bass as bass
import concourse.tile as tile
from concourse import bacc, mybir
from concourse.bass import IndirectOffsetOnAxis
from concourse.bass_utils import run_bass_kernel_spmd
from concourse.masks import make_identity

# Problem constants (hardcoded per spec)
N_CORES = 8
E_FULL = 250000
N_NODES = 100000
D = 128          # node/edge feature dim == LN dim
H = 512          # hidden dim
LN_EPS = 1e-5

G = 7            # 128-edge tiles per supertile
EC = 31360       # padded edges per core; EC = 245*128, 245 = 35*G
NT = EC // 128   # 245 edge tiles per core
NS = NT // G     # 35 supertiles per core

BF16 = mybir.dt.bfloat16
F32 = mybir.dt.float32
I32 = mybir.dt.int32

RSQRT_MAGIC = 0x5F3759DF

# transpose path: 'dma' = xbar DMA transpose (HWDGE), 'pe' = TensorE+identity
TP_MODE = "pe"

# bench bisection: 'full' | 'nogather' (skip indirect DMAs) | 'gatheronly'
# (skip transposes/matmuls/LN) | 'dmaonly' (skip gathers and compute)
VARIANT = "full"

_PROGRAM_CACHE = {}


def _rsqrt_batched(nc, stats, mg2, eps_negh):
    """inv = rsqrt(var + eps), nmi = -mu * inv, batched over the supertile.

    mg2: [128, 2G] f32 with (mean, var) pairs per tile; eps_negh unused slot.
    Returns (inv, nmi) [128, G] tiles. Uses the fp32 exponent-bit seed plus
    two Newton steps y <- y*(1.5 + (-veps/2)*y^2); rel err ~5e-6, plenty
    inside LN given bf16 matmul inputs.
    """
    mu = mg2[:, 0:2 * G:2]
    var = mg2[:, 1:2 * G:2]
    veps = stats.tile([128, G], F32, tag="veps")
    nc.vector.tensor_scalar(out=veps[:], in0=var, scalar1=LN_EPS, scalar2=None,
                            op0=mybir.AluOpType.add)
    hv = stats.tile([128, G], F32, tag="hv")
    nc.vector.tensor_scalar(out=hv[:], in0=veps[:], scalar1=-0.5, scalar2=None,
                            op0=mybir.AluOpType.mult)
    sh = stats.tile([128, G], I32, tag="sh")
    nc.vector.tensor_scalar(out=sh[:], in0=veps[:].bitcast(I32), scalar1=1,
                            scalar2=None,
                            op0=mybir.AluOpType.arith_shift_right)
    seed = stats.tile([128, G], I32, tag="seed")
    nc.vector.tensor_scalar(out=seed[:], in0=sh[:], scalar1=-1,
                            scalar2=RSQRT_MAGIC,
                            op0=mybir.AluOpType.mult,
                            op1=mybir.AluOpType.add)
    y = seed[:].bitcast(F32)
    for it in range(2):
        a = stats.tile([128, G], F32, tag=f"nr_a{it}")
        nc.vector.tensor_mul(out=a[:], in0=y, in1=y)
        b = stats.tile([128, G], F32, tag=f"nr_b{it}")
        nc.vector.tensor_mul(out=b[:], in0=a[:], in1=hv[:])
        ynew = stats.tile([128, G], F32, tag=f"nr_y{it}")
        nc.vector.scalar_tensor_tensor(out=ynew[:], in0=b[:], scalar=1.5,
                                       in1=y, op0=mybir.AluOpType.add,
                                       op1=mybir.AluOpType.mult)
        y = ynew[:]
    nmi = stats.tile([128, G], F32, tag="nmi")
    nc.vector.scalar_tensor_tensor(out=nmi[:], in0=mu, scalar=-1.0, in1=y,
                                   op0=mybir.AluOpType.mult,
                                   op1=mybir.AluOpType.mult)
    return y, nmi


def _build_program(trivial_affine: bool, sim_safe: bool = False,
                   repeats: int = 1):
    """Build (and cache) the Bass program. Returns the compiled Bacc.

    sim_safe=True replaces the Silu activation (not implemented in CoreSim)
    with Sigmoid + an explicit multiply; used only for simulator validation.
    repeats>1 wraps the whole body in a hardware For loop (benchmarking).
    """
    key = (trivial_affine, sim_safe, TP_MODE, repeats, VARIANT)
    if key in _PROGRAM_CACHE:
        return _PROGRAM_CACHE[key]
    do_gather = VARIANT in ("full", "gatheronly")
    do_compute = VARIANT in ("full", "nogather")

    nc = bacc.Bacc("TRN2", target_bir_lowering=False, debug=False,
                   num_devices=N_CORES)

    nodes = nc.dram_tensor("nodes", [2 * N_NODES, D], BF16, kind="ExternalInput").ap()
    edges = nc.dram_tensor("edges", [EC, D], BF16, kind="ExternalInput").ap()
    idx = nc.dram_tensor("idx", [EC, 2], I32, kind="ExternalInput").ap()
    w1 = nc.dram_tensor("w1", [D, 12 * D], BF16, kind="ExternalInput").ap()
    w2 = nc.dram_tensor("w2", [D, 4 * D], BF16, kind="ExternalInput").ap()
    out = nc.dram_tensor("out", [EC, D], F32, kind="ExternalOutput").ap()
    scratch = None
    if VARIANT in ("gatheronly", "dmaonly"):
        scratch = nc.dram_tensor("scratch", [2 * G * 128, D], BF16).ap()
    if not trivial_affine:
        b1d = nc.dram_tensor("b1d", [D, 4], F32, kind="ExternalInput").ap()
        b2d = nc.dram_tensor("b2d", [D, D], F32, kind="ExternalInput").ap()
        gmd = nc.dram_tensor("gmd", [D, D], F32, kind="ExternalInput").ap()
        btd = nc.dram_tensor("btd", [D, D], F32, kind="ExternalInput").ap()

    with tile.TileContext(nc) as tc, ExitStack() as ctx:
        const = ctx.enter_context(tc.tile_pool(name="const", bufs=1))
        io = ctx.enter_context(tc.tile_pool(name="io", bufs=3))
        idxp = ctx.enter_context(tc.tile_pool(name="idx", bufs=4))
        xtp = ctx.enter_context(tc.tile_pool(name="xt", bufs=3))
        htp = ctx.enter_context(tc.tile_pool(name="ht", bufs=3))
        stats = ctx.enter_context(tc.tile_pool(name="stats", bufs=2))
        ps_ht = ctx.enter_context(tc.tile_pool(name="ps_ht", bufs=3, space="PSUM"))
        ps_y = ctx.enter_context(tc.tile_pool(name="ps_y", bufs=3, space="PSUM"))
        if TP_MODE == "pe":
            ps_tp = ctx.enter_context(tc.tile_pool(name="ps_tp", bufs=2, space="PSUM"))

        # constants
        w1sb = const.tile([D, 12 * D], BF16)
        nc.sync.dma_start(out=w1sb[:], in_=w1[:])
        w2sb = const.tile([D, 4 * D], BF16)
        nc.sync.dma_start(out=w2sb[:], in_=w2[:])
        if TP_MODE == "pe":
            ident = const.tile([D, D], BF16)
            make_identity(nc, ident[:])
        if not trivial_affine:
            b1sb = const.tile([D, 4], F32)
            nc.sync.dma_start(out=b1sb[:], in_=b1d[:])
            b2sb = const.tile([D, D], F32)
            nc.sync.dma_start(out=b2sb[:], in_=b2d[:])
            gmsb = const.tile([D, D], F32)
            nc.sync.dma_start(out=gmsb[:], in_=gmd[:])
            btsb = const.tile([D, D], F32)
            nc.sync.dma_start(out=btsb[:], in_=btd[:])

        def _supertile(t):
            base = t * G * 128
            # combined src/dst indices: [p, 2g] = src edge(p,g), [p, 2g+1] = dst
            it_ = idxp.tile([128, 2 * G], I32, tag="idx")
            nc.sync.dma_start(
                out=it_[:],
                in_=idx[base:base + G * 128, :].rearrange("(p g) c -> p (g c)", g=G))
            sd = io.tile([128, 2 * G, D], BF16, tag="sd")
            # one [128,1]-index gather per column: walrus's indirect-DMA
            # lowering only agrees with the sim for one gathered row per
            # partition (multi-column offset APs fetch the wrong rows on HW)
            if do_gather:
                for k in range(2 * G):
                    nc.gpsimd.indirect_dma_start(
                        out=sd[:, k, :], out_offset=None, in_=nodes[:],
                        in_offset=IndirectOffsetOnAxis(ap=it_[:, k:k + 1], axis=0))
            else:
                # bisect probe: same bytes, plain contiguous DMA
                b2 = (t * 2 * G * 128) % (2 * N_NODES - 2 * G * 128)
                nc.sync.dma_start(
                    out=sd[:],
                    in_=nodes[b2:b2 + 2 * G * 128, :].rearrange(
                        "(p k) f -> p k f", k=2 * G))
            e_bf = io.tile([128, G, D], BF16, tag="e")
            nc.sync.dma_start(
                out=e_bf[:],
                in_=edges[base:base + G * 128, :].rearrange("(p g) f -> p g f", g=G))
            if do_compute:
                ysb = io.tile([128, G, D], F32, tag="ysb")
                yout = io.tile([128, G, D], F32, tag="yout")
                mg2 = stats.tile([128, 2 * G], F32, tag="mg2")

            for g in range(G if do_compute else 0):
                xt = xtp.tile([128, 3 * D], BF16)
                if TP_MODE == "dma":
                    nc.sync.dma_start(out=xt[:, 0:D], in_=sd[:, 2 * g, :], transpose=True)
                    nc.sync.dma_start(out=xt[:, D:2 * D], in_=sd[:, 2 * g + 1, :], transpose=True)
                    nc.sync.dma_start(out=xt[:, 2 * D:3 * D], in_=e_bf[:, g, :], transpose=True)
                else:
                    tp = ps_tp.tile([128, 3 * D], BF16)
                    nc.tensor.transpose(out=tp[:, 0:D], in_=sd[:, 2 * g, :], identity=ident[:])
                    nc.tensor.transpose(out=tp[:, D:2 * D], in_=sd[:, 2 * g + 1, :], identity=ident[:])
                    nc.tensor.transpose(out=tp[:, 2 * D:3 * D], in_=e_bf[:, g, :], identity=ident[:])
                    nc.vector.tensor_copy(out=xt[:], in_=tp[:])

                # mm1: hT[m-chunk partitions, edge free] for 4 chunks
                htps = ps_ht.tile([128, H], F32)
                for m in range(4):
                    for c in range(3):
                        nc.tensor.matmul(
                            out=htps[:, m * D:(m + 1) * D],
                            lhsT=w1sb[:, (c * 4 + m) * D:(c * 4 + m + 1) * D],
                            rhs=xt[:, c * D:(c + 1) * D],
                            start=(c == 0), stop=(c == 2))

                ht = htp.tile([128, H], BF16)
                if sim_safe:
                    sg = htp.tile([128, H], BF16, tag="sg")
                    nc.scalar.activation(out=sg[:], in_=htps[:],
                                         func=mybir.ActivationFunctionType.Sigmoid)
                    nc.vector.tensor_mul(out=ht[:], in0=htps[:], in1=sg[:])
                    if not trivial_affine:
                        raise NotImplementedError("sim_safe assumes trivial affine")
                elif trivial_affine:
                    nc.scalar.activation(out=ht[:], in_=htps[:],
                                         func=mybir.ActivationFunctionType.Silu)
                else:
                    for m in range(4):
                        nc.scalar.activation(
                            out=ht[:, m * D:(m + 1) * D],
                            in_=htps[:, m * D:(m + 1) * D],
                            func=mybir.ActivationFunctionType.Silu,
                            bias=b1sb[:, m:m + 1])

                # mm2: y[edge partitions, feature free]
                yps = ps_y.tile([128, D], F32)
                for m in range(4):
                    nc.tensor.matmul(
                        out=yps[:],
                        lhsT=ht[:, m * D:(m + 1) * D],
                        rhs=w2sb[:, m * D:(m + 1) * D],
                        start=(m == 0), stop=(m == 3))

                if not trivial_affine:
                    # y += b2 (pre-LN); write combined into ysb
                    nc.vector.tensor_add(out=ysb[:, g, :], in0=yps[:], in1=b2sb[:])
                else:
                    nc.scalar.activation(out=ysb[:, g, :], in_=yps[:],
                                         func=mybir.ActivationFunctionType.Copy)

                st6 = stats.tile([128, 6], F32, tag="st6")
                nc.vector.bn_stats(out=st6[:], in_=ysb[:, g, :])
                nc.vector.bn_aggr(out=mg2[:, 2 * g:2 * g + 2], in_=st6[:])

            if do_compute:
                inv, nmi = _rsqrt_batched(nc, stats, mg2, None)
            for g in range(G if do_compute else 0):
                if trivial_affine:
                    nc.vector.affine_then_add(
                        out=yout[:, g, :], in0=ysb[:, g, :], in1=e_bf[:, g, :],
                        scale=inv[:, g:g + 1], bias=nmi[:, g:g + 1])
                else:
                    yn = io.tile([128, D], F32, tag="yn")
                    nc.vector.tensor_scalar(out=yn[:], in0=ysb[:, g, :],
                                            scalar1=inv[:, g:g + 1],
                                            scalar2=nmi[:, g:g + 1],
                                            op0=mybir.AluOpType.mult,
                                            op1=mybir.AluOpType.add)
                    nc.vector.tensor_mul(out=yn[:], in0=yn[:], in1=gmsb[:])
                    nc.vector.tensor_add(out=yn[:], in0=yn[:], in1=btsb[:])
                    nc.vector.tensor_add(out=yout[:, g, :], in0=yn[:], in1=e_bf[:, g, :])

            if do_compute:
                nc.sync.dma_start(
                    out=out[base:base + G * 128, :].rearrange("(p g) f -> p g f", g=G),
                    in_=yout[:])
            else:
                nc.sync.dma_start(
                    out=scratch[:].rearrange("(p k) f -> p k f", k=2 * G),
                    in_=sd[:])

        if repeats == 1:
            for t in range(NS):
                _supertile(t)
        else:
            with tc.For_i(0, repeats, 1):
                for t in range(NS):
                    _supertile(t)

    nc.compile()
    _PROGRAM_CACHE[key] = nc
    return nc


def _prep(inputs):
    """Host-side preprocessing -> per-core input maps + metadata."""
    f = {k: np.asarray(v) for k, v in inputs.items()}
    bf = ml_dtypes.bfloat16

    nodes = np.concatenate([f["src_node_features"], f["dst_node_features"]],
                           axis=0).astype(bf)

    e = f["edge_features"].astype(np.float32)
    si = f["src_indices"].astype(np.int64)
    di = f["dst_indices"].astype(np.int64)
    E = e.shape[0]
    etot = EC * N_CORES
    e_pad = np.zeros((etot, D), np.float32)
    e_pad[:E] = e
    idx_pad = np.zeros((etot, 2), np.int32)
    idx_pad[:E, 0] = si.astype(np.int32)
    idx_pad[:E, 1] = (di + N_NODES).astype(np.int32)
    idx_pad[E:, 1] = N_NODES  # padding rows gather row 0 of each half
    e_bf = e_pad.astype(bf)

    W1 = f["W1"].astype(np.float32)
    W2 = f["W2"].astype(np.float32)
    w1b = np.concatenate(
        [W1[c * D:(c + 1) * D, m * D:(m + 1) * D] for c in range(3) for m in range(4)],
        axis=1).astype(bf)
    w2b = np.concatenate([W2[m * D:(m + 1) * D, :] for m in range(4)], axis=1).astype(bf)

    b1 = f["b1"].astype(np.float32)
    b2 = f["b2"].astype(np.float32)
    gm = f["ln_gamma"].astype(np.float32)
    bt = f["ln_beta"].astype(np.float32)
    trivial = (not b1.any()) and (not b2.any()) and (not bt.any()) and bool(np.all(gm == 1.0))

    in_maps = []
    for i in range(N_CORES):
        lo, hi = i * EC, (i + 1) * EC
        m = {
            "nodes": nodes,
            "edges": np.ascontiguousarray(e_bf[lo:hi]),
            "idx": np.ascontiguousarray(idx_pad[lo:hi]),
            "w1": w1b,
            "w2": w2b,
        }
        if not trivial:
            m["b1d"] = np.ascontiguousarray(b1.reshape(4, D).T.astype(np.float32))
            m["b2d"] = np.broadcast_to(b2, (D, D)).copy()
            m["gmd"] = np.broadcast_to(gm, (D, D)).copy()
            m["btd"] = np.broadcast_to(bt, (D, D)).copy()
        in_maps.append(m)
    return in_maps, trivial, E


def kernel(**inputs) -> np.ndarray:
    in_maps, trivial, E = _prep(inputs)
    nc = _build_program(trivial)
    res = run_bass_kernel_spmd(nc, in_maps, core_ids=list(range(N_CORES)))
    out = np.concatenate([res.results[i]["out"] for i in range(N_CORES)], axis=0)
    return np.ascontiguousarray(out[:E])



# revision 24
# speedup vs baseline: 1.1707x; 1.0968x over previous
"""Trainium2 Bass kernel for nn_MeshEdgeBlock (GNN edge-block message passing).

Computes, per edge e with endpoints (s, d):
    x  = concat([src_nodes[s], dst_nodes[d], edge_feat[e]])   # [384]
    h  = silu(x @ W1 + b1)                                    # [512]
    y  = h @ W2 + b2                                          # [128]
    y  = LayerNorm(y) * gamma + beta + edge_feat[e]           # [128]

Sharding: edges assigned to the 8 cores by SRC-NODE RANGE (12500 rows per
core); within a core, edges are bucketed by dst-table chunk (4 chunks of
25000 rows, the int16 dma_gather index range) and sorted by src id.

Gather strategy: the two per-edge random gathers are split asymmetrically.
 - dst side: GpSimd dma_gather in transpose mode, ONE op per 2048 edges
   (single queue, single_packet=False - the only configuration measured
   correct on HW; all groups padded to exactly 16 tiles so every gather is
   exactly 2048 rows).
 - src side: NO random DMA at all. Because edges are sorted by src id, each
   128-edge tile's src rows span < NB*128 consecutive table rows. The host
   uploads that window's CONTENT per tile (contiguous DMA at full rate) plus
   the in-window offset loc[e]; on-chip, a one-hot matrix built from
   iota/is_equal on VectorE selects the rows via NB accumulating matmuls on
   TensorE (gather-as-matmul), producing src features already transposed
   [feat, edge] for mm1.

Device-side dataflow per group (16 tiles of 128 edges):
  - 1 dma_gather (dst) -> dT [128 feat, 2048 edges] bf16
  - edge features: plain DMA (residual) + xbar DMA transpose -> eT (mm1)
  - per tile: window DMA -> wsb [128 row, NB, 128 feat]; loc broadcast
    (GpSimd partition_broadcast); NB is_equal one-hots (VectorE); NB
    matmuls -> sxT psum -> sx bf16 (ScalarE copy); mm1 12 matmuls; silu;
    mm2 4 matmuls; LN stats via bn_stats/bn_aggr
  - rsqrt(var+eps) once per group (exponent-bit seed + 2 Newton steps);
    normalize + residual fused in one affine_then_add; bf16 out written
    p-major, un-permuted on host
"""

import numpy as np
import ml_dtypes
from contextlib import ExitStack

import concourse.bass as bass
import concourse.tile as tile
from concourse import bacc, library_config, mybir
from concourse.bass_utils import run_bass_kernel_spmd

# Problem constants (hardcoded per spec)
N_CORES = 8
E_FULL = 250000
N_NODES = 100000
D = 128          # node/edge feature dim == LN dim
H = 512          # hidden dim
LN_EPS = 1e-5

SRC_R = N_NODES // N_CORES   # 12500 src rows per core
DCH = 25000                  # dst-table chunk rows (int16 gather range)
NDC = 4                      # dst chunks
GT = 16                      # tiles per group (gather = GT*128 = 2048 rows)

BF16 = mybir.dt.bfloat16
F32 = mybir.dt.float32
I32 = mybir.dt.int32
I16 = mybir.dt.int16

RSQRT_MAGIC = 0x5F3759DF

_PROGRAM_CACHE = {}
_LAYOUT = {}


def _rsqrt_batched(nc, stats, mg2, gt):
    """inv = rsqrt(var+eps), nmi = -mu*inv, batched over the group."""
    mu = mg2[:, 0:2 * gt:2]
    var = mg2[:, 1:2 * gt:2]
    veps = stats.tile([128, GT], F32, tag="veps")
    nc.vector.tensor_scalar(out=veps[:, :gt], in0=var, scalar1=LN_EPS,
                            scalar2=None, op0=mybir.AluOpType.add)
    hv = stats.tile([128, GT], F32, tag="hv")
    nc.vector.tensor_scalar(out=hv[:, :gt], in0=veps[:, :gt], scalar1=-0.5,
                            scalar2=None, op0=mybir.AluOpType.mult)
    sh = stats.tile([128, GT], I32, tag="sh")
    nc.vector.tensor_scalar(out=sh[:, :gt], in0=veps[:, :gt].bitcast(I32),
                            scalar1=1, scalar2=None,
                            op0=mybir.AluOpType.arith_shift_right)
    seed = stats.tile([128, GT], I32, tag="seed")
    nc.vector.tensor_scalar(out=seed[:, :gt], in0=sh[:, :gt], scalar1=-1,
                            scalar2=RSQRT_MAGIC,
                            op0=mybir.AluOpType.mult,
                            op1=mybir.AluOpType.add)
    y = seed[:, :gt].bitcast(F32)
    for it in range(2):
        a = stats.tile([128, GT], F32, tag=f"nr_a{it}")
        nc.vector.tensor_mul(out=a[:, :gt], in0=y, in1=y)
        b = stats.tile([128, GT], F32, tag=f"nr_b{it}")
        nc.vector.tensor_mul(out=b[:, :gt], in0=a[:, :gt], in1=hv[:, :gt])
        ynew = stats.tile([128, GT], F32, tag=f"nr_y{it}")
        nc.vector.scalar_tensor_tensor(out=ynew[:, :gt], in0=b[:, :gt],
                                       scalar=1.5, in1=y,
                                       op0=mybir.AluOpType.add,
                                       op1=mybir.AluOpType.mult)
        y = ynew[:, :gt]
    nmi = stats.tile([128, GT], F32, tag="nmi")
    nc.vector.scalar_tensor_tensor(out=nmi[:, :gt], in0=mu, scalar=-1.0,
                                   in1=y, op0=mybir.AluOpType.mult,
                                   op1=mybir.AluOpType.mult)
    return y, nmi[:, :gt]


def _build_program(trivial_affine: bool, repeats: int = 1):
    """Build (and cache) the Bass program for the layout in _LAYOUT."""
    NB = _LAYOUT["NB"]
    t_b = tuple(_LAYOUT["t_b"])      # tiles per dst-chunk bucket (mult of GT)
    key = (trivial_affine, repeats, NB, t_b)
    if key in _PROGRAM_CACHE:
        return _PROGRAM_CACHE[key]

    NT = sum(t_b)
    NG = NT // GT
    EC2 = NT * 128

    nc = bacc.Bacc("TRN2", target_bir_lowering=False, debug=False,
                   num_devices=N_CORES)

    dnodes = nc.dram_tensor("dnodes", [N_NODES, D], BF16, kind="ExternalInput").ap()
    edges = nc.dram_tensor("edges", [EC2, D], BF16, kind="ExternalInput").ap()
    didx = nc.dram_tensor("didx", [NG * 128, GT * 8], I16, kind="ExternalInput").ap()
    win = nc.dram_tensor("win", [NG * 128, GT * NB * D], BF16, kind="ExternalInput").ap()
    locd = nc.dram_tensor("locd", [NG, GT * 128], F32, kind="ExternalInput").ap()
    iot = nc.dram_tensor("iot", [128, NB], F32, kind="ExternalInput").ap()
    w1 = nc.dram_tensor("w1", [D, 12 * D], BF16, kind="ExternalInput").ap()
    w2 = nc.dram_tensor("w2", [D, 4 * D], BF16, kind="ExternalInput").ap()
    out = nc.dram_tensor("out", [EC2, D], BF16, kind="ExternalOutput").ap()
    if not trivial_affine:
        b1d = nc.dram_tensor("b1d", [D, 4], F32, kind="ExternalInput").ap()
        b2d = nc.dram_tensor("b2d", [D, D], F32, kind="ExternalInput").ap()
        gmd = nc.dram_tensor("gmd", [D, D], F32, kind="ExternalInput").ap()
        btd = nc.dram_tensor("btd", [D, D], F32, kind="ExternalInput").ap()

    with tile.TileContext(nc) as tc, ExitStack() as ctx:
        const = ctx.enter_context(tc.tile_pool(name="const", bufs=1))
        io = ctx.enter_context(tc.tile_pool(name="io", bufs=3))
        idxp = ctx.enter_context(tc.tile_pool(name="idx", bufs=3))
        gtp = ctx.enter_context(tc.tile_pool(name="gt", bufs=3))
        wp = ctx.enter_context(tc.tile_pool(name="wp", bufs=3))
        ohp = ctx.enter_context(tc.tile_pool(name="oh", bufs=3))
        htp = ctx.enter_context(tc.tile_pool(name="ht", bufs=3))
        stats = ctx.enter_context(tc.tile_pool(name="stats", bufs=2))
        ps_sx = ctx.enter_context(tc.tile_pool(name="ps_sx", bufs=2, space="PSUM"))
        ps_ht = ctx.enter_context(tc.tile_pool(name="ps_ht", bufs=3, space="PSUM"))
        ps_y = ctx.enter_context(tc.tile_pool(name="ps_y", bufs=3, space="PSUM"))

        # dma_gather lives in the dynamically-loaded 'mlp' Q7 library
        nc.gpsimd.load_library(library_config.mlp)

        w1sb = const.tile([D, 12 * D], BF16)
        nc.sync.dma_start(out=w1sb[:], in_=w1[:])
        w2sb = const.tile([D, 4 * D], BF16)
        nc.sync.dma_start(out=w2sb[:], in_=w2[:])
        iots = const.tile([128, NB], F32)
        nc.sync.dma_start(out=iots[:], in_=iot[:])
        if not trivial_affine:
            b1sb = const.tile([D, 4], F32)
            nc.sync.dma_start(out=b1sb[:], in_=b1d[:])
            b2sb = const.tile([D, D], F32)
            nc.sync.dma_start(out=b2sb[:], in_=b2d[:])
            gmsb = const.tile([D, D], F32)
            nc.sync.dma_start(out=gmsb[:], in_=gmd[:])
            btsb = const.tile([D, D], F32)
            nc.sync.dma_start(out=btsb[:], in_=btd[:])

        def _group(g, dc, toff):
            base = toff * 128
            ne = GT * 128
            it_d = idxp.tile([128, GT * 8], I16, tag="didx")
            nc.sync.dma_start(out=it_d[:], in_=didx[g * 128:(g + 1) * 128, :])
            dT = gtp.tile([128, 1, GT * 128], BF16, tag="dT")
            nc.gpsimd.dma_gather(
                dT[:], dnodes[dc * DCH:(dc + 1) * DCH, :],
                it_d[:], ne, ne, D, transpose=True, single_packet=False)
            e_bf = io.tile([128, GT, D], BF16, tag="e")
            nc.sync.dma_start(
                out=e_bf[:],
                in_=edges[base:base + ne, :].rearrange("(g p) f -> p g f", p=128))
            eT = gtp.tile([128, GT * 128], BF16, tag="eT")
            nc.sync.dma_start(out=eT[:], in_=edges[base:base + ne, :],
                              transpose=True)
            loc_sb = idxp.tile([1, GT * 128], F32, tag="loc")
            nc.sync.dma_start(out=loc_sb[:], in_=locd[g:g + 1, :])
            # whole group's src windows, host-laid p-major: one contiguous
            # read per partition
            wg = wp.tile([128, GT, NB, D], BF16, tag="wsb")
            nc.sync.dma_start(
                out=wg[:].rearrange("p t b f -> p (t b f)"),
                in_=win[g * 128:(g + 1) * 128, :])
            ysb = io.tile([128, GT, D], F32, tag="ysb")
            yout = io.tile([128, GT, D], BF16, tag="yout")
            mg2 = stats.tile([128, 2 * GT], F32, tag="mg2")

            for t in range(GT):
                T = toff + t
                co = t * 128
                # one-hot selection: oh_k[r, e] = (loc[e] == k*128 + r)
                locB = ohp.tile([128, D], F32, tag="locB")
                nc.gpsimd.partition_broadcast(
                    locB[:], loc_sb[0:1, t * 128:(t + 1) * 128])
                oh = ohp.tile([128, NB, D], BF16, tag="oh")
                for k in range(NB):
                    nc.vector.tensor_scalar(out=oh[:, k, :], in0=locB[:],
                                            scalar1=iots[:, k:k + 1],
                                            scalar2=None,
                                            op0=mybir.AluOpType.is_equal)
                sxps = ps_sx.tile([128, D], F32)
                for k in range(NB):
                    nc.tensor.matmul(out=sxps[:], lhsT=wg[:, t, k, :],
                                     rhs=oh[:, k, :],
                                     start=(k == 0), stop=(k == NB - 1))
                sx = wp.tile([128, D], BF16, tag="sx")
                nc.scalar.activation(out=sx[:], in_=sxps[:],
                                     func=mybir.ActivationFunctionType.Copy)

                rhs3 = (sx[:], dT[:, 0, co:co + 128], eT[:, co:co + 128])
                htps = ps_ht.tile([128, H], F32)
                for m in range(4):
                    for c in range(3):
                        nc.tensor.matmul(
                            out=htps[:, m * D:(m + 1) * D],
                            lhsT=w1sb[:, (c * 4 + m) * D:(c * 4 + m + 1) * D],
                            rhs=rhs3[c],
                            start=(c == 0), stop=(c == 2))

                ht = htp.tile([128, H], BF16)
                if trivial_affine:
                    nc.scalar.activation(out=ht[:], in_=htps[:],
                                         func=mybir.ActivationFunctionType.Silu)
                else:
                    for m in range(4):
                        nc.scalar.activation(
                            out=ht[:, m * D:(m + 1) * D],
                            in_=htps[:, m * D:(m + 1) * D],
                            func=mybir.ActivationFunctionType.Silu,
                            bias=b1sb[:, m:m + 1])

                yps = ps_y.tile([128, D], F32)
                for m in range(4):
                    nc.tensor.matmul(
                        out=yps[:],
                        lhsT=ht[:, m * D:(m + 1) * D],
                        rhs=w2sb[:, m * D:(m + 1) * D],
                        start=(m == 0), stop=(m == 3))

                if not trivial_affine:
                    nc.vector.tensor_add(out=ysb[:, t, :], in0=yps[:], in1=b2sb[:])
                else:
                    nc.scalar.activation(out=ysb[:, t, :], in_=yps[:],
                                         func=mybir.ActivationFunctionType.Copy)

                st6 = stats.tile([128, 6], F32, tag="st6")
                nc.vector.bn_stats(out=st6[:], in_=ysb[:, t, :])
                nc.vector.bn_aggr(out=mg2[:, 2 * t:2 * t + 2], in_=st6[:])

            inv, nmi = _rsqrt_batched(nc, stats, mg2[:, :2 * GT], GT)
            for t in range(GT):
                if trivial_affine:
                    nc.vector.affine_then_add(
                        out=yout[:, t, :], in0=ysb[:, t, :], in1=e_bf[:, t, :],
                        scale=inv[:, t:t + 1], bias=nmi[:, t:t + 1])
                else:
                    yn = io.tile([128, D], F32, tag="yn")
                    nc.vector.tensor_scalar(out=yn[:], in0=ysb[:, t, :],
                                            scalar1=inv[:, t:t + 1],
                                            scalar2=nmi[:, t:t + 1],
                                            op0=mybir.AluOpType.mult,
                                            op1=mybir.AluOpType.add)
                    nc.vector.tensor_mul(out=yn[:], in0=yn[:], in1=gmsb[:])
                    nc.vector.tensor_add(out=yn[:], in0=yn[:], in1=btsb[:])
                    nc.vector.tensor_add(out=yout[:, t, :], in0=yn[:], in1=e_bf[:, t, :])

            # p-major row order (row = p*GT + t): contiguous per-partition
            # writes; the host un-permutes via perms
            nc.sync.dma_start(
                out=out[base:base + ne, :].rearrange("(p g) f -> p g f", g=GT),
                in_=yout[:])

        def _body():
            toff = 0
            g = 0
            for dc in range(NDC):
                for _ in range(t_b[dc] // GT):
                    _group(g, dc, toff)
                    toff += GT
                    g += 1

        if repeats == 1:
            _body()
        else:
            with tc.For_i(0, repeats, 1):
                _body()

    nc.compile()
    _PROGRAM_CACHE[key] = nc
    return nc


def _prep(inputs):
    """Host-side src-range sharding, dst bucketing, window construction."""
    f = {k: np.asarray(v) for k, v in inputs.items()}
    bf = ml_dtypes.bfloat16

    src_bf = f["src_node_features"].astype(bf)
    dst_bf = f["dst_node_features"].astype(bf)
    e_bf = f["edge_features"].astype(bf)
    si = f["src_indices"].astype(np.int64)
    di = f["dst_indices"].astype(np.int64)
    E = e_bf.shape[0]

    core_of = si // SRC_R
    dc_of = di // DCH

    # per (core, dc): edge id lists sorted by src id
    lists = []
    maxspan = 0
    for core in range(N_CORES):
        row = []
        sel = np.nonzero(core_of == core)[0]
        for dc in range(NDC):
            m = sel[dc_of[sel] == dc]
            m = m[np.argsort(si[m], kind="stable")]
            row.append(m)
            # track max 128-edge tile span for NB sizing
            sl = si[m] - core * SRC_R
            for p0 in range(0, len(sl), 128):
                seg = sl[p0:p0 + 128]
                if len(seg) > 1:
                    maxspan = max(maxspan, int(seg[-1] - seg[0]))
        lists.append(row)

    NB = max(3, -(-(maxspan + 1) // 128))
    t_b = []
    for dc in range(NDC):
        mx = max(len(lists[c][dc]) for c in range(N_CORES))
        tb = -(-mx // 128)
        tb = -(-tb // GT) * GT            # round up to whole groups
        t_b.append(tb)
    NT = sum(t_b)
    NG = NT // GT
    EC2 = NT * 128
    _LAYOUT["NB"] = NB
    _LAYOUT["t_b"] = t_b

    W1 = f["W1"].astype(np.float32)
    W2 = f["W2"].astype(np.float32)
    w1b = np.concatenate(
        [W1[c * D:(c + 1) * D, m * D:(m + 1) * D] for c in range(3) for m in range(4)],
        axis=1).astype(bf)
    w2b = np.concatenate([W2[m * D:(m + 1) * D, :] for m in range(4)], axis=1).astype(bf)

    b1 = f["b1"].astype(np.float32)
    b2 = f["b2"].astype(np.float32)
    gm = f["ln_gamma"].astype(np.float32)
    bt = f["ln_beta"].astype(np.float32)
    trivial = (not b1.any()) and (not b2.any()) and (not bt.any()) and bool(np.all(gm == 1.0))

    iota = np.zeros((128, NB), np.float32)
    for k in range(NB):
        iota[:, k] = np.arange(128) + k * 128

    in_maps, perms = [], []
    for core in range(N_CORES):
        slab = np.vstack([src_bf[core * SRC_R:(core + 1) * SRC_R],
                          np.zeros((NB * 128, D), bf)])
        e_core = np.zeros((EC2, D), bf)
        p_core = np.full((EC2,), -1, np.int64)
        dl_core = np.zeros((EC2,), np.int16)
        winb = np.zeros((NT, NB, 128, D), bf)
        locb = np.zeros((NT, 128), np.float32)
        toff = 0
        for dc in range(NDC):
            lst = lists[core][dc]
            pos = 0
            for tl in range(t_b[dc]):
                T = toff + tl
                base = T * 128
                seg = lst[pos:pos + 128]
                pos += 128
                k = len(seg)
                wb = 0
                if k:
                    sl = si[seg] - core * SRC_R
                    wb = int(sl[0])
                    locb[T, :k] = sl - wb
                    e_core[base:base + k] = e_bf[seg]
                    dl_core[base:base + k] = (di[seg] - dc * DCH).astype(np.int16)
                    # out rows are p-major within the group: row =
                    # group_base + lane*GT + tile_in_group
                    gb = (T // GT) * GT * 128
                    ti = T % GT
                    p_core[gb + np.arange(k) * GT + ti] = seg
                winb[T] = slab[wb:wb + NB * 128].reshape(NB, 128, D)
            toff += t_b[dc]

        # wrap dst indices into dma_gather's 16-partition layout, replicated
        # 8x down the 128 partitions (one copy per Q7 core)
        didx_b = np.zeros((NG, 128, GT * 8), np.int16)
        for g in range(NG):
            blk = dl_core[g * GT * 128:(g + 1) * GT * 128]
            didx_b[g] = np.tile(blk.reshape(GT * 8, 16).T, (8, 1))

        m = {
            "dnodes": dst_bf,
            "edges": e_core,
            "didx": didx_b.reshape(NG * 128, GT * 8),
            "win": np.ascontiguousarray(
                winb.reshape(NG, GT, NB, 128, D).transpose(0, 3, 1, 2, 4)
            ).reshape(NG * 128, GT * NB * D),
            "locd": locb.reshape(NG, GT * 128),
            "iot": iota,
            "w1": w1b,
            "w2": w2b,
        }
        if not trivial:
            m["b1d"] = np.ascontiguousarray(b1.reshape(4, D).T.astype(np.float32))
            m["b2d"] = np.broadcast_to(b2, (D, D)).copy()
            m["gmd"] = np.broadcast_to(gm, (D, D)).copy()
            m["btd"] = np.broadcast_to(bt, (D, D)).copy()
        in_maps.append(m)
        perms.append(p_core)
    return in_maps, trivial, perms


def kernel(**inputs) -> np.ndarray:
    in_maps, trivial, perms = _prep(inputs)
    nc = _build_program(trivial)
    res = run_bass_kernel_spmd(nc, in_maps, core_ids=list(range(N_CORES)))
    E = np.asarray(inputs["edge_features"]).shape[0]
    out = np.empty((E, D), np.float32)
    for core in range(N_CORES):
        o = np.asarray(res.results[core]["out"]).astype(np.float32)
        p = perms[core]
        valid = p >= 0
        out[p[valid]] = o[valid]
    return out


# revision 25
# speedup vs baseline: 1.2082x; 1.0321x over previous
"""Trainium2 Bass kernel for nn_MeshEdgeBlock (GNN edge-block message passing).

Computes, per edge e with endpoints (s, d):
    x  = concat([src_nodes[s], dst_nodes[d], edge_feat[e]])   # [384]
    h  = silu(x @ W1 + b1)                                    # [512]
    y  = h @ W2 + b2                                          # [128]
    y  = LayerNorm(y) * gamma + beta + edge_feat[e]           # [128]

Sharding: edges assigned to the 8 cores by SRC-NODE RANGE (12500 rows per
core); within a core, edges are bucketed by dst-table chunk (4 chunks of
25000 rows, the int16 dma_gather index range) and sorted by src id.

Gather strategy: the two per-edge random gathers are split asymmetrically.
 - dst side: GpSimd dma_gather in transpose mode, ONE op per 2048 edges
   (single queue, single_packet=False - the only configuration measured
   correct on HW; all groups padded to exactly 16 tiles so every gather is
   exactly 2048 rows).
 - src side: NO random DMA at all. Because edges are sorted by src id, each
   128-edge tile's src rows span < NB*128 consecutive table rows. The host
   uploads that window's CONTENT per tile (contiguous DMA at full rate) plus
   the in-window offset loc[e]; on-chip, a one-hot matrix built from
   iota/is_equal on VectorE selects the rows via NB accumulating matmuls on
   TensorE (gather-as-matmul), producing src features already transposed
   [feat, edge] for mm1.

Device-side dataflow per group (16 tiles of 128 edges):
  - 1 dma_gather (dst) -> dT [128 feat, 2048 edges] bf16
  - edge features: plain DMA (residual) + xbar DMA transpose -> eT (mm1)
  - per tile: window DMA -> wsb [128 row, NB, 128 feat]; loc broadcast
    (GpSimd partition_broadcast); NB is_equal one-hots (VectorE); NB
    matmuls -> sxT psum -> sx bf16 (ScalarE copy); mm1 12 matmuls; silu;
    mm2 4 matmuls; LN stats via bn_stats/bn_aggr
  - rsqrt(var+eps) once per group (exponent-bit seed + 2 Newton steps);
    normalize + residual fused in one affine_then_add; bf16 out written
    p-major, un-permuted on host
"""

import numpy as np
import ml_dtypes
from contextlib import ExitStack

import concourse.bass as bass
import concourse.tile as tile
from concourse import bacc, library_config, mybir
from concourse.bass_utils import run_bass_kernel_spmd

# Problem constants (hardcoded per spec)
N_CORES = 8
E_FULL = 250000
N_NODES = 100000
D = 128          # node/edge feature dim == LN dim
H = 512          # hidden dim
LN_EPS = 1e-5

SRC_R = N_NODES // N_CORES   # 12500 src rows per core
DCH = 25000                  # dst-table chunk rows (int16 gather range)
NDC = 4                      # dst chunks
GT = 16                      # tiles per group (gather = GT*128 = 2048 rows)

BF16 = mybir.dt.bfloat16
F32 = mybir.dt.float32
I32 = mybir.dt.int32
I16 = mybir.dt.int16

RSQRT_MAGIC = 0x5F3759DF

_PROGRAM_CACHE = {}
_LAYOUT = {}


def _rsqrt_batched(nc, stats, mg2, gt):
    """inv = rsqrt(var+eps), nmi = -mu*inv, batched over the group."""
    mu = mg2[:, 0:2 * gt:2]
    var = mg2[:, 1:2 * gt:2]
    veps = stats.tile([128, GT], F32, tag="veps")
    nc.vector.tensor_scalar(out=veps[:, :gt], in0=var, scalar1=LN_EPS,
                            scalar2=None, op0=mybir.AluOpType.add)
    hv = stats.tile([128, GT], F32, tag="hv")
    nc.vector.tensor_scalar(out=hv[:, :gt], in0=veps[:, :gt], scalar1=-0.5,
                            scalar2=None, op0=mybir.AluOpType.mult)
    sh = stats.tile([128, GT], I32, tag="sh")
    nc.vector.tensor_scalar(out=sh[:, :gt], in0=veps[:, :gt].bitcast(I32),
                            scalar1=1, scalar2=None,
                            op0=mybir.AluOpType.arith_shift_right)
    seed = stats.tile([128, GT], I32, tag="seed")
    nc.vector.tensor_scalar(out=seed[:, :gt], in0=sh[:, :gt], scalar1=-1,
                            scalar2=RSQRT_MAGIC,
                            op0=mybir.AluOpType.mult,
                            op1=mybir.AluOpType.add)
    y = seed[:, :gt].bitcast(F32)
    for it in range(2):
        a = stats.tile([128, GT], F32, tag=f"nr_a{it}")
        nc.vector.tensor_mul(out=a[:, :gt], in0=y, in1=y)
        b = stats.tile([128, GT], F32, tag=f"nr_b{it}")
        nc.vector.tensor_mul(out=b[:, :gt], in0=a[:, :gt], in1=hv[:, :gt])
        ynew = stats.tile([128, GT], F32, tag=f"nr_y{it}")
        nc.vector.scalar_tensor_tensor(out=ynew[:, :gt], in0=b[:, :gt],
                                       scalar=1.5, in1=y,
                                       op0=mybir.AluOpType.add,
                                       op1=mybir.AluOpType.mult)
        y = ynew[:, :gt]
    nmi = stats.tile([128, GT], F32, tag="nmi")
    nc.vector.scalar_tensor_tensor(out=nmi[:, :gt], in0=mu, scalar=-1.0,
                                   in1=y, op0=mybir.AluOpType.mult,
                                   op1=mybir.AluOpType.mult)
    return y, nmi[:, :gt]


def _build_program(trivial_affine: bool, repeats: int = 1):
    """Build (and cache) the Bass program for the layout in _LAYOUT."""
    NB = _LAYOUT["NB"]
    t_b = tuple(_LAYOUT["t_b"])      # tiles per dst-chunk bucket (mult of GT)
    key = (trivial_affine, repeats, NB, t_b)
    if key in _PROGRAM_CACHE:
        return _PROGRAM_CACHE[key]

    NT = sum(t_b)
    NG = NT // GT
    EC2 = NT * 128

    nc = bacc.Bacc("TRN2", target_bir_lowering=False, debug=False,
                   num_devices=N_CORES)

    dnodes = nc.dram_tensor("dnodes", [N_NODES, D], BF16, kind="ExternalInput").ap()
    edges = nc.dram_tensor("edges", [EC2, D], BF16, kind="ExternalInput").ap()
    didx = nc.dram_tensor("didx", [NG * 128, GT * 8], I16, kind="ExternalInput").ap()
    win = nc.dram_tensor("win", [NG * 128, GT * NB * D], BF16, kind="ExternalInput").ap()
    locd = nc.dram_tensor("locd", [NG, GT * 128], F32, kind="ExternalInput").ap()
    iot = nc.dram_tensor("iot", [128, NB], F32, kind="ExternalInput").ap()
    w1 = nc.dram_tensor("w1", [D, 12 * D], BF16, kind="ExternalInput").ap()
    w2 = nc.dram_tensor("w2", [D, 4 * D], BF16, kind="ExternalInput").ap()
    out = nc.dram_tensor("out", [EC2, D], BF16, kind="ExternalOutput").ap()
    if not trivial_affine:
        b1d = nc.dram_tensor("b1d", [D, 4], F32, kind="ExternalInput").ap()
        b2d = nc.dram_tensor("b2d", [D, D], F32, kind="ExternalInput").ap()
        gmd = nc.dram_tensor("gmd", [D, D], F32, kind="ExternalInput").ap()
        btd = nc.dram_tensor("btd", [D, D], F32, kind="ExternalInput").ap()

    with tile.TileContext(nc) as tc, ExitStack() as ctx:
        const = ctx.enter_context(tc.tile_pool(name="const", bufs=1))
        io = ctx.enter_context(tc.tile_pool(name="io", bufs=4))
        idxp = ctx.enter_context(tc.tile_pool(name="idx", bufs=5))
        gtp = ctx.enter_context(tc.tile_pool(name="gt", bufs=5))
        wp = ctx.enter_context(tc.tile_pool(name="wp", bufs=3))
        ohp = ctx.enter_context(tc.tile_pool(name="oh", bufs=3))
        htp = ctx.enter_context(tc.tile_pool(name="ht", bufs=3))
        stats = ctx.enter_context(tc.tile_pool(name="stats", bufs=2))
        ps_sx = ctx.enter_context(tc.tile_pool(name="ps_sx", bufs=2, space="PSUM"))
        ps_ht = ctx.enter_context(tc.tile_pool(name="ps_ht", bufs=3, space="PSUM"))
        ps_y = ctx.enter_context(tc.tile_pool(name="ps_y", bufs=3, space="PSUM"))

        # dma_gather lives in the dynamically-loaded 'mlp' Q7 library
        nc.gpsimd.load_library(library_config.mlp)

        w1sb = const.tile([D, 12 * D], BF16)
        nc.sync.dma_start(out=w1sb[:], in_=w1[:])
        w2sb = const.tile([D, 4 * D], BF16)
        nc.sync.dma_start(out=w2sb[:], in_=w2[:])
        iots = const.tile([128, NB], F32)
        nc.sync.dma_start(out=iots[:], in_=iot[:])
        if not trivial_affine:
            b1sb = const.tile([D, 4], F32)
            nc.sync.dma_start(out=b1sb[:], in_=b1d[:])
            b2sb = const.tile([D, D], F32)
            nc.sync.dma_start(out=b2sb[:], in_=b2d[:])
            gmsb = const.tile([D, D], F32)
            nc.sync.dma_start(out=gmsb[:], in_=gmd[:])
            btsb = const.tile([D, D], F32)
            nc.sync.dma_start(out=btsb[:], in_=btd[:])

        def _group(g, dc, toff):
            base = toff * 128
            ne = GT * 128
            it_d = idxp.tile([128, GT * 8], I16, tag="didx")
            nc.sync.dma_start(out=it_d[:], in_=didx[g * 128:(g + 1) * 128, :])
            dT = gtp.tile([128, 1, GT * 128], BF16, tag="dT")
            nc.gpsimd.dma_gather(
                dT[:], dnodes[dc * DCH:(dc + 1) * DCH, :],
                it_d[:], ne, ne, D, transpose=True, single_packet=False)
            e_bf = io.tile([128, GT, D], BF16, tag="e")
            nc.sync.dma_start(
                out=e_bf[:],
                in_=edges[base:base + ne, :].rearrange("(g p) f -> p g f", p=128))
            eT = gtp.tile([128, GT * 128], BF16, tag="eT")
            nc.sync.dma_start(out=eT[:], in_=edges[base:base + ne, :],
                              transpose=True)
            loc_sb = idxp.tile([1, GT * 128], F32, tag="loc")
            nc.sync.dma_start(out=loc_sb[:], in_=locd[g:g + 1, :])
            # whole group's src windows, host-laid p-major: one contiguous
            # read per partition
            wg = wp.tile([128, GT, NB, D], BF16, tag="wsb")
            nc.sync.dma_start(
                out=wg[:].rearrange("p t b f -> p (t b f)"),
                in_=win[g * 128:(g + 1) * 128, :])
            ysb = io.tile([128, GT, D], F32, tag="ysb")
            yout = io.tile([128, GT, D], BF16, tag="yout")
            mg2 = stats.tile([128, 2 * GT], F32, tag="mg2")

            for t in range(GT):
                T = toff + t
                co = t * 128
                # one-hot selection: oh_k[r, e] = (loc[e] == k*128 + r)
                locB = ohp.tile([128, D], F32, tag="locB")
                nc.gpsimd.partition_broadcast(
                    locB[:], loc_sb[0:1, t * 128:(t + 1) * 128])
                oh = ohp.tile([128, NB, D], BF16, tag="oh")
                for k in range(NB):
                    nc.vector.tensor_scalar(out=oh[:, k, :], in0=locB[:],
                                            scalar1=iots[:, k:k + 1],
                                            scalar2=None,
                                            op0=mybir.AluOpType.is_equal)
                sxps = ps_sx.tile([128, D], F32)
                for k in range(NB):
                    nc.tensor.matmul(out=sxps[:], lhsT=wg[:, t, k, :],
                                     rhs=oh[:, k, :],
                                     start=(k == 0), stop=(k == NB - 1))
                sx = wp.tile([128, D], BF16, tag="sx")
                nc.scalar.activation(out=sx[:], in_=sxps[:],
                                     func=mybir.ActivationFunctionType.Copy)

                rhs3 = (sx[:], dT[:, 0, co:co + 128], eT[:, co:co + 128])
                htps = ps_ht.tile([128, H], F32)
                for m in range(4):
                    for c in range(3):
                        nc.tensor.matmul(
                            out=htps[:, m * D:(m + 1) * D],
                            lhsT=w1sb[:, (c * 4 + m) * D:(c * 4 + m + 1) * D],
                            rhs=rhs3[c],
                            start=(c == 0), stop=(c == 2))

                ht = htp.tile([128, H], BF16)
                if trivial_affine:
                    nc.scalar.activation(out=ht[:], in_=htps[:],
                                         func=mybir.ActivationFunctionType.Silu)
                else:
                    for m in range(4):
                        nc.scalar.activation(
                            out=ht[:, m * D:(m + 1) * D],
                            in_=htps[:, m * D:(m + 1) * D],
                            func=mybir.ActivationFunctionType.Silu,
                            bias=b1sb[:, m:m + 1])

                yps = ps_y.tile([128, D], F32)
                for m in range(4):
                    nc.tensor.matmul(
                        out=yps[:],
                        lhsT=ht[:, m * D:(m + 1) * D],
                        rhs=w2sb[:, m * D:(m + 1) * D],
                        start=(m == 0), stop=(m == 3))

                if not trivial_affine:
                    nc.vector.tensor_add(out=ysb[:, t, :], in0=yps[:], in1=b2sb[:])
                else:
                    nc.scalar.activation(out=ysb[:, t, :], in_=yps[:],
                                         func=mybir.ActivationFunctionType.Copy)

                st6 = stats.tile([128, 6], F32, tag="st6")
                nc.vector.bn_stats(out=st6[:], in_=ysb[:, t, :])
                nc.vector.bn_aggr(out=mg2[:, 2 * t:2 * t + 2], in_=st6[:])

            inv, nmi = _rsqrt_batched(nc, stats, mg2[:, :2 * GT], GT)
            for t in range(GT):
                if trivial_affine:
                    nc.vector.affine_then_add(
                        out=yout[:, t, :], in0=ysb[:, t, :], in1=e_bf[:, t, :],
                        scale=inv[:, t:t + 1], bias=nmi[:, t:t + 1])
                else:
                    yn = io.tile([128, D], F32, tag="yn")
                    nc.vector.tensor_scalar(out=yn[:], in0=ysb[:, t, :],
                                            scalar1=inv[:, t:t + 1],
                                            scalar2=nmi[:, t:t + 1],
                                            op0=mybir.AluOpType.mult,
                                            op1=mybir.AluOpType.add)
                    nc.vector.tensor_mul(out=yn[:], in0=yn[:], in1=gmsb[:])
                    nc.vector.tensor_add(out=yn[:], in0=yn[:], in1=btsb[:])
                    nc.vector.tensor_add(out=yout[:, t, :], in0=yn[:], in1=e_bf[:, t, :])

            # p-major row order (row = p*GT + t): contiguous per-partition
            # writes; the host un-permutes via perms
            nc.sync.dma_start(
                out=out[base:base + ne, :].rearrange("(p g) f -> p g f", g=GT),
                in_=yout[:])

        def _body():
            toff = 0
            g = 0
            for dc in range(NDC):
                for _ in range(t_b[dc] // GT):
                    _group(g, dc, toff)
                    toff += GT
                    g += 1

        if repeats == 1:
            _body()
        else:
            with tc.For_i(0, repeats, 1):
                _body()

    nc.compile()
    _PROGRAM_CACHE[key] = nc
    return nc


def _prep(inputs):
    """Host-side src-range sharding, dst bucketing, window construction."""
    f = {k: np.asarray(v) for k, v in inputs.items()}
    bf = ml_dtypes.bfloat16

    src_bf = f["src_node_features"].astype(bf)
    dst_bf = f["dst_node_features"].astype(bf)
    e_bf = f["edge_features"].astype(bf)
    si = f["src_indices"].astype(np.int64)
    di = f["dst_indices"].astype(np.int64)
    E = e_bf.shape[0]

    core_of = si // SRC_R
    dc_of = di // DCH

    # per (core, dc): edge id lists sorted by src id
    lists = []
    maxspan = 0
    for core in range(N_CORES):
        row = []
        sel = np.nonzero(core_of == core)[0]
        for dc in range(NDC):
            m = sel[dc_of[sel] == dc]
            m = m[np.argsort(si[m], kind="stable")]
            row.append(m)
            # track max 128-edge tile span for NB sizing
            sl = si[m] - core * SRC_R
            for p0 in range(0, len(sl), 128):
                seg = sl[p0:p0 + 128]
                if len(seg) > 1:
                    maxspan = max(maxspan, int(seg[-1] - seg[0]))
        lists.append(row)

    NB = max(3, -(-(maxspan + 1) // 128))
    t_b = []
    for dc in range(NDC):
        mx = max(len(lists[c][dc]) for c in range(N_CORES))
        tb = -(-mx // 128)
        tb = -(-tb // GT) * GT            # round up to whole groups
        t_b.append(tb)
    NT = sum(t_b)
    NG = NT // GT
    EC2 = NT * 128
    _LAYOUT["NB"] = NB
    _LAYOUT["t_b"] = t_b

    W1 = f["W1"].astype(np.float32)
    W2 = f["W2"].astype(np.float32)
    w1b = np.concatenate(
        [W1[c * D:(c + 1) * D, m * D:(m + 1) * D] for c in range(3) for m in range(4)],
        axis=1).astype(bf)
    w2b = np.concatenate([W2[m * D:(m + 1) * D, :] for m in range(4)], axis=1).astype(bf)

    b1 = f["b1"].astype(np.float32)
    b2 = f["b2"].astype(np.float32)
    gm = f["ln_gamma"].astype(np.float32)
    bt = f["ln_beta"].astype(np.float32)
    trivial = (not b1.any()) and (not b2.any()) and (not bt.any()) and bool(np.all(gm == 1.0))

    iota = np.zeros((128, NB), np.float32)
    for k in range(NB):
        iota[:, k] = np.arange(128) + k * 128

    in_maps, perms = [], []
    for core in range(N_CORES):
        slab = np.vstack([src_bf[core * SRC_R:(core + 1) * SRC_R],
                          np.zeros((NB * 128, D), bf)])
        e_core = np.zeros((EC2, D), bf)
        p_core = np.full((EC2,), -1, np.int64)
        dl_core = np.zeros((EC2,), np.int16)
        winb = np.zeros((NT, NB, 128, D), bf)
        locb = np.zeros((NT, 128), np.float32)
        toff = 0
        for dc in range(NDC):
            lst = lists[core][dc]
            pos = 0
            for tl in range(t_b[dc]):
                T = toff + tl
                base = T * 128
                seg = lst[pos:pos + 128]
                pos += 128
                k = len(seg)
                wb = 0
                if k:
                    sl = si[seg] - core * SRC_R
                    wb = int(sl[0])
                    locb[T, :k] = sl - wb
                    e_core[base:base + k] = e_bf[seg]
                    dl_core[base:base + k] = (di[seg] - dc * DCH).astype(np.int16)
                    # out rows are p-major within the group: row =
                    # group_base + lane*GT + tile_in_group
                    gb = (T // GT) * GT * 128
                    ti = T % GT
                    p_core[gb + np.arange(k) * GT + ti] = seg
                winb[T] = slab[wb:wb + NB * 128].reshape(NB, 128, D)
            toff += t_b[dc]

        # wrap dst indices into dma_gather's 16-partition layout, replicated
        # 8x down the 128 partitions (one copy per Q7 core)
        didx_b = np.zeros((NG, 128, GT * 8), np.int16)
        for g in range(NG):
            blk = dl_core[g * GT * 128:(g + 1) * GT * 128]
            didx_b[g] = np.tile(blk.reshape(GT * 8, 16).T, (8, 1))

        m = {
            "dnodes": dst_bf,
            "edges": e_core,
            "didx": didx_b.reshape(NG * 128, GT * 8),
            "win": np.ascontiguousarray(
                winb.reshape(NG, GT, NB, 128, D).transpose(0, 3, 1, 2, 4)
            ).reshape(NG * 128, GT * NB * D),
            "locd": locb.reshape(NG, GT * 128),
            "iot": iota,
            "w1": w1b,
            "w2": w2b,
        }
        if not trivial:
            m["b1d"] = np.ascontiguousarray(b1.reshape(4, D).T.astype(np.float32))
            m["b2d"] = np.broadcast_to(b2, (D, D)).copy()
            m["gmd"] = np.broadcast_to(gm, (D, D)).copy()
            m["btd"] = np.broadcast_to(bt, (D, D)).copy()
        in_maps.append(m)
        perms.append(p_core)
    return in_maps, trivial, perms


def kernel(**inputs) -> np.ndarray:
    in_maps, trivial, perms = _prep(inputs)
    nc = _build_program(trivial)
    res = run_bass_kernel_spmd(nc, in_maps, core_ids=list(range(N_CORES)))
    E = np.asarray(inputs["edge_features"]).shape[0]
    out = np.empty((E, D), np.float32)
    for core in range(N_CORES):
        o = np.asarray(res.results[core]["out"]).astype(np.float32)
        p = perms[core]
        valid = p >= 0
        out[p[valid]] = o[valid]
    return out
